# revision 7
# baseline (speedup 1.0000x reference)
"""Trainium2 Bass kernel for masked multi-modal causal dot-product attention.

Computation (reference):
  Q = mlp(x1, Wq)               # (4096, 64), 3 linear layers, relu between
  for m in 0..3:
    K_m = mlp(x_m, Wk[m])       # (4096, 64)
    mask_m[i,j] = t2_m[j] <= t1[i]   (timestamps sorted -> staircase mask)
    acc += ((Q @ K_m.T) * mask_m) @ x_m[:, :2]
  out = acc  # (1, 4096, 2)

Sharding: 8 cores = 4 modalities x 2 contiguous query halves (2048 queries
each). One SPMD program; per-core variation lives in the input tensors.

Key algebraic optimization: for key tiles FULLY visible to a whole query
block, ((Q K^T) * 1) V = Q (K^T V). Per 128-key pair tile j we form
G_j^T = V_j^T K_j (2x64) with two tiny matmuls:
  - probe:  sp = kTblk_j^T @ [I64; I64]  -> K values, keys on partitions
  - reduce: G_j^T = xkv_j^T @ sp         -> PSUM (2, 64)
G tiles are staged to SBUF and DMA'd into a [32, 2, 64] partition-stacked
array; a step-vector matmul (host-built, per-core data) then selects the
prefix sum G_pref_b = sum_{j < F[b]} G_j for each query block -> the whole
fully-visible region costs ONE 512-col matmul per block. Only the ~5 boundary
tiles per block (keys whose timestamp falls inside the block's time span) run
the explicit S -> fused mask-multiply (DVE scalar_tensor_tensor) -> AV path.
Boundary keys are host-gathered into fixed tile slots so a single program
serves all cores; padded slots use t2=+inf and mask to zero.

Packing (from baseline): feature dim 64 is packed to contraction 128
everywhere (block-diagonal MLP weights on stacked halves, block-diagonal
K^T pair tiles, Q^T replicated onto both partition halves). All matmuls f32r.
"""

import os
import sys

import numpy as np

sys.path.insert(0, "/opt/trn_rl_repo")

T = 4096
D = 64
M = 4
NLIN = 3
NQ = 2048          # queries per core (contiguous half)
CHUNK = 128        # keys per pair tile (64 even + 64 odd)
NPAIR = T // CHUNK  # 32 sorted pair tiles
IBLK = 512         # query block (moving dim)
NBLK = NQ // IBLK  # 4 query blocks per core

LAST_RESULTS = None


def _build_program(NBB):
    """NBB[b]: boundary slots for query block b (same for all cores; per-core
    variation is in the gathered input data)."""
    import concourse.bacc as bacc
    import concourse.mybir as mybir
    import concourse.tile as tile

    f32 = mybir.dt.float32
    f32r = mybir.dt.float32r
    Relu = mybir.ActivationFunctionType.Relu
    Identity = mybir.ActivationFunctionType.Identity
    is_ge = mybir.AluOpType.is_ge
    add = mybir.AluOpType.add
    amax = mybir.AluOpType.max
    mult = mybir.AluOpType.mult

    NBSLOT = sum(NBB)             # total boundary slots
    NSLOT = NPAIR + NBSLOT        # total pair tiles in kTblk
    KCOLS = NSLOT * 64            # K-MLP moving columns
    boff = [NPAIR + sum(NBB[:b]) for b in range(NBLK)]  # first slot of block b

    nc = bacc.Bacc("TRN2", target_bir_lowering=False, debug=False, num_devices=8)

    xqT = nc.dram_tensor("xqT", [128, NQ // 2], f32, kind="ExternalInput")
    xkT = nc.dram_tensor("xkT", [128, KCOLS], f32, kind="ExternalInput")
    xkv = nc.dram_tensor("xkv", [128, NSLOT * 2], f32, kind="ExternalInput")
    xt2b = nc.dram_tensor("xt2b", [128, max(NBSLOT, 1)], f32, kind="ExternalInput")
    t1p = nc.dram_tensor("t1p", [1, NQ], f32, kind="ExternalInput")
    probe = nc.dram_tensor("probe", [128, 64], f32, kind="ExternalInput")
    stepm = nc.dram_tensor("stepm", [NPAIR, NBLK], f32, kind="ExternalInput")
    wq = nc.dram_tensor("wq", [128, 4 * 128], f32, kind="ExternalInput")
    bq = nc.dram_tensor("bq", [128, 4], f32, kind="ExternalInput")
    wk = nc.dram_tensor("wk", [128, NLIN * 128], f32, kind="ExternalInput")
    bk = nc.dram_tensor("bk", [128, NLIN], f32, kind="ExternalInput")
    out = nc.dram_tensor("out", [2, NQ], f32, kind="ExternalOutput")

    def rr(ap):
        return ap.bitcast(f32r)

    with tile.TileContext(nc) as tc:
        with (
            tc.tile_pool(name="const", bufs=1) as const,
            tc.tile_pool(name="hq", bufs=2) as hqp,
            tc.tile_pool(name="hk", bufs=2) as hkp,
            tc.tile_pool(name="spool", bufs=NBSLOT) as spool,
            tc.tile_pool(name="gpool", bufs=3) as gpool,
            tc.tile_pool(name="gstg", bufs=2) as gstg,
            tc.tile_pool(name="ps_a", bufs=3, space="PSUM") as ps_a,
            tc.tile_pool(name="ps_s", bufs=3, space="PSUM") as ps_s,
            tc.tile_pool(name="ps_og", bufs=2, space="PSUM") as ps_og,
        ):
            # ---- inputs -> SBUF (weights first, x chunked for overlap)
            wk_sb = const.tile([128, NLIN, 128], f32r)
            nc.scalar.dma_start(wk_sb[:], rr(wk[:]).rearrange("p (l e) -> p l e", l=NLIN))
            wq_sb = const.tile([128, 4, 128], f32r)
            nc.scalar.dma_start(wq_sb[:], rr(wq[:]).rearrange("p (l e) -> p l e", l=4))
            bk_sb = const.tile([128, NLIN], f32)
            nc.scalar.dma_start(bk_sb[:], bk[:])
            bq_sb = const.tile([128, 4], f32)
            nc.scalar.dma_start(bq_sb[:], bq[:])
            xkv_sb = const.tile([128, NSLOT, 2], f32r)
            nc.gpsimd.dma_start(xkv_sb[:], rr(xkv[:]).rearrange("p (c f) -> p c f", f=2))
            xt2b_sb = const.tile([128, max(NBSLOT, 1)], f32)
            nc.gpsimd.dma_start(xt2b_sb[:], xt2b[:])
            probe_sb = const.tile([128, 64], f32r)
            nc.gpsimd.dma_start(probe_sb[:], rr(probe[:]))
            step_sb = const.tile([NPAIR, NBLK], f32r)
            nc.gpsimd.dma_start(step_sb[:], rr(stepm[:]))

            xqT_sb = const.tile([128, NQ // 2], f32r)
            for nb in range(NQ // 2 // IBLK):
                sl = slice(nb * IBLK, (nb + 1) * IBLK)
                nc.scalar.dma_start(xqT_sb[:, sl], rr(xqT[:, sl]))
            t1b_sb = const.tile([CHUNK, NQ], f32)
            nc.scalar.dma_start(t1b_sb[:], t1p[:].partition_broadcast(CHUNK))
            xkT_sb = const.tile([128, KCOLS], f32r)
            nchk = -(-KCOLS // IBLK)
            for nb in range(nchk):
                sl = slice(nb * IBLK, min((nb + 1) * IBLK, KCOLS))
                nc.sync.dma_start(xkT_sb[:, sl], rr(xkT[:, sl]))

            out_sb = const.tile([2, NQ], f32)

            # ---- blocked K^T target: pair tiles with block-diagonal layout
            kTblk = const.tile([128, NSLOT, CHUNK], f32r)
            nc.gpsimd.memset(kTblk[0:64, :, 64:128].bitcast(f32), 0.0)
            nc.gpsimd.memset(kTblk[64:128, :, 0:64].bitcast(f32), 0.0)
            qT2 = const.tile([128, NQ], f32r)
            G_stack = const.tile([NPAIR, 2, 64], f32r)
            gstat = const.tile([128, NBLK, 2], f32r)
            nc.gpsimd.memset(gstat[:].bitcast(f32), 0.0)

            # ---- stacked MLPs (block-diagonal weights, both halves at once)
            def epilogue(dst, ps, bias, layer, eng):
                if eng == "act":
                    func = Relu if layer < NLIN - 1 else Identity
                    nc.scalar.activation(dst, ps, func, bias=bias)
                elif layer < NLIN - 1:
                    nc.vector.tensor_scalar(dst, ps, bias, 0.0, op0=add, op1=amax)
                else:
                    nc.vector.tensor_scalar(dst, ps, bias, None, op0=add)

            def mlp_hidden(cur, w_sb, b_sb, pool, nt, layer, eng):
                nxt = pool.tile([128, nt], f32r, tag="h")
                for nb in range(-(-nt // IBLK)):
                    sl = slice(nb * IBLK, min((nb + 1) * IBLK, nt))
                    csz = sl.stop - sl.start
                    ps = ps_a.tile([128, csz], f32, tag="a")
                    nc.tensor.matmul(
                        ps[:], w_sb[:, layer, :], cur[:, sl], start=True, stop=True
                    )
                    epilogue(nxt[:, sl], ps[:], b_sb[:, layer : layer + 1], layer, eng)
                return nxt

            hk, hq = xkT_sb, xqT_sb
            for layer in range(NLIN - 1):
                hk = mlp_hidden(hk, wk_sb, bk_sb, hkp, KCOLS, layer, "act")
                hq = mlp_hidden(hq, wq_sb, bq_sb, hqp, NQ // 2, layer, "dve")

            # final K layer: write straight into block-diagonal pair tiles.
            # Boundary chunks (slots >= NPAIR) first so the boundary S+mask
            # phase can start early; sorted chunks later, overlapping the
            # DVE mask drain.
            eng_flip = 0

            def k_final_chunk(nb):
                nonlocal eng_flip
                sl = slice(nb * IBLK, min((nb + 1) * IBLK, KCOLS))
                csz = sl.stop - sl.start
                ps = ps_a.tile([128, csz], f32, tag="a", name="ps")
                nc.tensor.matmul(
                    ps[:], wk_sb[:, NLIN - 1, :], hk[:, sl], start=True, stop=True
                )
                psv = ps[:].rearrange("p (a e) -> p a e", e=64)
                pair = slice(8 * nb, 8 * nb + csz // 64)
                bias = bk_sb[:, NLIN - 1 : NLIN]
                for half, csl in ((slice(0, 64), slice(0, 64)),
                                  (slice(64, 128), slice(64, 128))):
                    dst = kTblk[half, pair, csl]
                    srcv = psv[half, :, :]
                    if eng_flip % 2 == 0:
                        nc.scalar.activation(dst, srcv, Identity, bias=bias[half])
                    else:
                        nc.vector.tensor_scalar(dst, srcv, bias[half], None, op0=add)
                    eng_flip += 1

            nbchunk = NPAIR // 8  # first chunk holding boundary slots
            for nb in range(nbchunk, nchk):
                k_final_chunk(nb)

            # final Q layer: replicate Q^T onto both partition halves
            for nb in range(NQ // 2 // IBLK):
                sl = slice(nb * IBLK, (nb + 1) * IBLK)
                bias = bq_sb[:, NLIN - 1 : NLIN]
                for rep in range(2):
                    ps = ps_a.tile([128, IBLK], f32, tag="a")
                    nc.tensor.matmul(
                        ps[:], wq_sb[:, 2 + rep, :], hq[:, sl], start=True, stop=True
                    )
                    osl = slice(rep * (NQ // 2) + nb * IBLK,
                                rep * (NQ // 2) + (nb + 1) * IBLK)
                    epilogue(qT2[:, osl], ps[:], bias, NLIN - 1,
                             "act" if rep else "dve")

            # ---- boundary S + mask phase (early): masked S kept in SBUF so
            # the DVE mask work overlaps the probe/G phase on the PE.
            s_tiles = {}
            for b in range(NBLK):
                isl = slice(b * IBLK, (b + 1) * IBLK)
                for s in range(NBB[b]):
                    slot = boff[b] + s
                    bidx = slot - NPAIR
                    sp = ps_s.tile([CHUNK, IBLK], f32, tag="s", name="sp")
                    nc.tensor.matmul(
                        sp[:], kTblk[:, slot, :], qT2[:, isl],
                        start=True, stop=True, skip_group_check=True,
                    )
                    s_sb = spool.tile([CHUNK, IBLK], f32r, name="s_sb")
                    nc.vector.scalar_tensor_tensor(
                        s_sb[:], t1b_sb[:, isl], xt2b_sb[:, bidx:bidx + 1], sp[:],
                        op0=is_ge, op1=mult,
                    )
                    s_tiles[(b, s)] = s_sb

            # sorted K-final chunks (kTblk slots 0..NPAIR-1) for the G phase
            for nb in range(nbchunk):
                k_final_chunk(nb)

            # ---- G phase: G_j^T = V_j^T K_j via probe matmuls, 8 tiles/round
            # (probe-S rounds batched into one [128,512] PSUM tile + one ACT
            # copy; probe-AV rounds deferred one round so the copy overlaps
            # the next probe-S round on the PE)
            def emit_gav(rr_, sps_):
                gps = ps_og.tile([2, 512], f32, tag="og", name="gps")
                for slq in range(8):
                    j = rr_ * 8 + slq
                    nc.tensor.matmul(
                        gps[:, slq * 64:(slq + 1) * 64], xkv_sb[:, j, :],
                        sps_[:, slq * 64:(slq + 1) * 64],
                        start=True, stop=True, skip_group_check=True,
                    )
                gst = gstg.tile([2, 512], f32r, name="gst")
                nc.vector.tensor_copy(gst[:], gps[:])
                for c in range(2):
                    nc.gpsimd.dma_start(
                        G_stack[rr_ * 8:(rr_ + 1) * 8, c, :], gst[c:c + 1, :]
                    )

            prevg = None
            for r in range(NPAIR // 8):
                spb = ps_a.tile([128, 512], f32, tag="a", name="spb")
                for slq in range(8):
                    j = r * 8 + slq
                    nc.tensor.matmul(
                        spb[:, slq * 64:(slq + 1) * 64], kTblk[:, j, :],
                        probe_sb[:], start=True, stop=True, skip_group_check=True,
                    )
                sps = gpool.tile([128, 512], f32r, name="sps")
                nc.scalar.copy(sps[:], spb[:])
                if prevg is not None:
                    emit_gav(*prevg)
                prevg = (r, sps)
            emit_gav(*prevg)

            # ---- prefix select: G_pref_b = sum_{j < F[b]} G_j (step is data!)
            psel = ps_s.tile([64, 2 * NBLK], f32, tag="s")
            for c in range(2):
                nc.tensor.matmul(
                    psel[:, c * NBLK:(c + 1) * NBLK], G_stack[:, c, :], step_sb[:],
                    start=True, stop=True, skip_group_check=True,
                )
            for b in range(NBLK):
                for c in range(2):
                    i = c * NBLK + b
                    nc.scalar.copy(gstat[0:64, b, c:c + 1], psel[:, i:i + 1])

            # ---- output accumulation: full-region matmul + boundary AVs
            for b in range(NBLK):
                isl = slice(b * IBLK, (b + 1) * IBLK)
                ov = ps_og.tile([2, IBLK], f32, tag="og", name="ov")
                nc.tensor.matmul(
                    ov[:], gstat[:, b, :], qT2[:, isl],
                    start=True, stop=False, skip_group_check=True,
                )
                for s in range(NBB[b]):
                    slot = boff[b] + s
                    nc.tensor.matmul(
                        ov[:], xkv_sb[:, slot, :], s_tiles[(b, s)][:],
                        start=False, stop=(s == NBB[b] - 1),
                        skip_group_check=True,
                    )
                nc.scalar.copy(out_sb[:, isl], ov[:])

            nc.sync.dma_start(out[:], out_sb[:])

    nc.compile()
    return nc


def kernel(x1, x2, x3, x4, Wq_w, Wq_b, Wk_w, Wk_b):
    from concourse.bass_utils import run_bass_kernel_spmd

    global LAST_RESULTS

    xs = [np.asarray(a, dtype=np.float32)[0, 0] for a in (x1, x2, x3, x4)]
    Wq_w = np.asarray(Wq_w, dtype=np.float32)
    Wq_b = np.asarray(Wq_b, dtype=np.float32)
    Wk_w = np.asarray(Wk_w, dtype=np.float32)
    Wk_b = np.asarray(Wk_b, dtype=np.float32)

    t1 = xs[0][:, -1]
    t2s = [x[:, -1] for x in xs]

    # ---- per-core full/boundary classification (exact, from timestamps)
    FJ = {}  # (m, p) -> (F[b], J[b])
    NBB = [1] * NBLK
    for p in range(2):
        qoff = NQ * p
        for m in range(M):
            F, J = [], []
            for b in range(NBLK):
                lo = t1[qoff + b * IBLK]
                hi = t1[qoff + b * IBLK + IBLK - 1]
                nfull = int(np.searchsorted(t2s[m], lo, side="right"))
                nvis = int(np.searchsorted(t2s[m], hi, side="right"))
                F.append(nfull // CHUNK)
                J.append(-(-nvis // CHUNK))
                NBB[b] = max(NBB[b], J[b] - F[b])
            FJ[(m, p)] = (F, J)

    nc = _build_program(NBB)

    NBSLOT = sum(NBB)
    boff = [sum(NBB[:b]) for b in range(NBLK)]

    # ---- host packing
    def blockdiag(Wl):
        b = np.zeros((128, 128), np.float32)
        b[:64, :64] = Wl
        b[64:, 64:] = Wl
        return b

    # Q weights: layers 0,1 blockdiag; final as [[W,W],[0,0]] and [[0,0],[W,W]]
    wq_h = np.zeros((4, 128, 128), np.float32)
    for l in range(NLIN - 1):
        wq_h[l] = blockdiag(Wq_w[l])
    wq_h[2, :64, :64] = Wq_w[2]
    wq_h[2, :64, 64:] = Wq_w[2]
    wq_h[3, 64:, :64] = Wq_w[2]
    wq_h[3, 64:, 64:] = Wq_w[2]
    wq_h = np.ascontiguousarray(wq_h.transpose(1, 0, 2).reshape(128, 4 * 128))
    bq_h = np.tile(Wq_b.T, (2, 1))  # [128, 3]
    bq_h = np.ascontiguousarray(
        np.concatenate([bq_h, bq_h[:, 2:3]], axis=1)
    )  # [128, 4]

    probe_h = np.ascontiguousarray(
        np.concatenate([np.eye(64, dtype=np.float32)] * 2, axis=0)
    )  # [128, 64]

    x1T = np.ascontiguousarray(xs[0].T)

    def pack_tile(xrows):
        """[128, D] key rows -> ([128, 64] xkT block, [128, 2] V, [128] t2)."""
        ev, od = xrows[0:64], xrows[64:128]
        blk = np.concatenate([ev.T, od.T], axis=0)  # [128, 64]
        v = np.concatenate([ev[:, 0:2], od[:, 0:2]], axis=0)  # [128, 2]
        tt = np.concatenate([ev[:, -1], od[:, -1]], axis=0)  # [128]
        return blk, v, tt

    in_maps = []
    for c in range(8):
        m, p = c // 2, c % 2
        xm = xs[m]
        qoff = NQ * p
        F, J = FJ[(m, p)]

        NSLOT = NPAIR + NBSLOT
        xkT_h = np.zeros((128, NSLOT * 64), np.float32)
        xkv_h = np.zeros((128, NSLOT, 2), np.float32)
        xt2b_h = np.full((128, max(NBSLOT, 1)), 1e30, np.float32)
        for j in range(NPAIR):
            blk, v, tt = pack_tile(xm[CHUNK * j:CHUNK * (j + 1)])
            xkT_h[:, 64 * j:64 * (j + 1)] = blk
            xkv_h[:, j] = v
        for b in range(NBLK):
            for s in range(NBB[b]):
                t = F[b] + s
                slot = NPAIR + boff[b] + s
                if t < J[b]:
                    blk, v, tt = pack_tile(xm[CHUNK * t:CHUNK * (t + 1)])
                    xkT_h[:, 64 * slot:64 * (slot + 1)] = blk
                    xkv_h[:, slot] = v
                    xt2b_h[:, boff[b] + s] = tt
        step_h = np.zeros((NPAIR, NBLK), np.float32)
        for b in range(NBLK):
            step_h[: F[b], b] = 1.0

        wk_h = np.stack([blockdiag(Wk_w[m][l]) for l in range(NLIN)])
        wk_h = np.ascontiguousarray(wk_h.transpose(1, 0, 2).reshape(128, NLIN * 128))
        bk_h = np.ascontiguousarray(np.tile(Wk_b[m].T, (2, 1)))  # [128, 3]

        # query-side: contiguous half, [first 1024 | second 1024] stacking
        xq = x1T[:, qoff:qoff + NQ]  # [64, 2048]
        xqT_h = np.concatenate([xq[:, : NQ // 2], xq[:, NQ // 2:]], axis=0)

        in_maps.append(
            {
                "xqT": np.ascontiguousarray(xqT_h),
                "xkT": xkT_h,
                "xkv": np.ascontiguousarray(xkv_h.reshape(128, NSLOT * 2)),
                "xt2b": xt2b_h,
                "t1p": np.ascontiguousarray(t1[qoff:qoff + NQ][None, :]),
                "probe": probe_h,
                "stepm": step_h,
                "wq": wq_h,
                "bq": bq_h,
                "wk": wk_h,
                "bk": bk_h,
            }
        )

    res = run_bass_kernel_spmd(nc, in_maps, core_ids=list(range(8)))
    LAST_RESULTS = res

    # ---- gather: sum over modalities per contiguous half, transpose
    acc = np.zeros((2, T), dtype=np.float32)
    for c in range(8):
        m, p = c // 2, c % 2
        acc[:, NQ * p:NQ * (p + 1)] += res.results[c]["out"]
    return np.ascontiguousarray(acc.T)[None]


# revision 8
# speedup vs baseline: 1.0979x; 1.0979x over previous
"""Trainium2 Bass kernel for masked multi-modal causal dot-product attention.

Computation (reference):
  Q = mlp(x1, Wq)               # (4096, 64), 3 linear layers, relu between
  for m in 0..3:
    K_m = mlp(x_m, Wk[m])       # (4096, 64)
    mask_m[i,j] = t2_m[j] <= t1[i]   (timestamps sorted -> staircase mask)
    acc += ((Q @ K_m.T) * mask_m) @ x_m[:, :2]
  out = acc  # (1, 4096, 2)

Sharding: 8 cores = 4 modalities x 2 contiguous query halves (2048 queries
each). One SPMD program; per-core variation lives in the input tensors.

Key algebraic optimization: for key tiles FULLY visible to a whole query
block, ((Q K^T) * 1) V = Q (K^T V). Per 128-key pair tile j we form
G_j^T = V_j^T K_j (2x64) with two tiny matmuls:
  - probe:  sp = kTblk_j^T @ [I64; I64]  -> K values, keys on partitions
  - reduce: G_j^T = xkv_j^T @ sp         -> PSUM (2, 64)
G tiles are staged to SBUF and DMA'd into a [32, 2, 64] partition-stacked
array; a step-vector matmul (host-built, per-core data) then selects the
prefix sum G_pref_b = sum_{j < F[b]} G_j for each query block -> the whole
fully-visible region costs ONE 512-col matmul per block. Only the ~5 boundary
tiles per block (keys whose timestamp falls inside the block's time span) run
the explicit S -> fused mask-multiply (DVE scalar_tensor_tensor) -> AV path.
Boundary keys are host-gathered into fixed tile slots so a single program
serves all cores; padded slots use t2=+inf and mask to zero.

Packing (from baseline): feature dim 64 is packed to contraction 128
everywhere (block-diagonal MLP weights on stacked halves, block-diagonal
K^T pair tiles, Q^T replicated onto both partition halves). All matmuls f32r.
"""

import os
import sys

import numpy as np
import ml_dtypes

BF16 = ml_dtypes.bfloat16

sys.path.insert(0, "/opt/trn_rl_repo")

T = 4096
D = 64
M = 4
NLIN = 3
NQ = 2048          # queries per core (contiguous half)
CHUNK = 128        # keys per pair tile (64 even + 64 odd)
NPAIR = T // CHUNK  # 32 sorted pair tiles
IBLK = 512         # query block (moving dim)
NBLK = NQ // IBLK  # 4 query blocks per core

LAST_RESULTS = None


def _build_program(NBB):
    """NBB[b]: boundary slots for query block b (same for all cores; per-core
    variation is in the gathered input data)."""
    import concourse.bacc as bacc
    import concourse.mybir as mybir
    import concourse.tile as tile

    f32 = mybir.dt.float32
    f32r = mybir.dt.float32r
    bf16 = mybir.dt.bfloat16
    Relu = mybir.ActivationFunctionType.Relu
    Identity = mybir.ActivationFunctionType.Identity
    is_ge = mybir.AluOpType.is_ge
    add = mybir.AluOpType.add
    amax = mybir.AluOpType.max
    mult = mybir.AluOpType.mult

    NBSLOT = sum(NBB)             # total boundary slots
    NSLOT = NPAIR + NBSLOT        # total pair tiles in kTblk
    KCOLS = NSLOT * 64            # K-MLP moving columns
    boff = [NPAIR + sum(NBB[:b]) for b in range(NBLK)]  # first slot of block b

    nc = bacc.Bacc("TRN2", target_bir_lowering=False, debug=False, num_devices=8)

    xqT = nc.dram_tensor("xqT", [128, NQ // 2], bf16, kind="ExternalInput")
    xkT = nc.dram_tensor("xkT", [128, KCOLS], bf16, kind="ExternalInput")
    xkv = nc.dram_tensor("xkv", [128, NSLOT * 2], bf16, kind="ExternalInput")
    xt2b = nc.dram_tensor("xt2b", [128, max(NBSLOT, 1)], f32, kind="ExternalInput")
    t1p = nc.dram_tensor("t1p", [1, NQ], f32, kind="ExternalInput")
    probe = nc.dram_tensor("probe", [128, 64], bf16, kind="ExternalInput")
    stepm = nc.dram_tensor("stepm", [NPAIR, NBLK], bf16, kind="ExternalInput")
    wq = nc.dram_tensor("wq", [128, 4 * 128], bf16, kind="ExternalInput")
    bq = nc.dram_tensor("bq", [128, 4], f32, kind="ExternalInput")
    wk = nc.dram_tensor("wk", [128, NLIN * 128], bf16, kind="ExternalInput")
    bk = nc.dram_tensor("bk", [128, NLIN], f32, kind="ExternalInput")
    out = nc.dram_tensor("out", [2, NQ], f32, kind="ExternalOutput")

    def rr(ap):
        return ap.bitcast(f32r)

    with tile.TileContext(nc) as tc:
        with (
            tc.tile_pool(name="const", bufs=1) as const,
            tc.tile_pool(name="hq", bufs=2) as hqp,
            tc.tile_pool(name="hk", bufs=2) as hkp,
            tc.tile_pool(name="spool", bufs=NBSLOT) as spool,
            tc.tile_pool(name="gpool", bufs=3) as gpool,
            tc.tile_pool(name="gstg", bufs=2) as gstg,
            tc.tile_pool(name="ps_a", bufs=3, space="PSUM") as ps_a,
            tc.tile_pool(name="ps_s", bufs=3, space="PSUM") as ps_s,
            tc.tile_pool(name="ps_og", bufs=2, space="PSUM") as ps_og,
        ):
            # ---- inputs -> SBUF (weights first, x chunked for overlap)
            wk_sb = const.tile([128, NLIN, 128], bf16)
            nc.scalar.dma_start(wk_sb[:], wk[:].rearrange("p (l e) -> p l e", l=NLIN))
            wq_sb = const.tile([128, 4, 128], bf16)
            nc.scalar.dma_start(wq_sb[:], wq[:].rearrange("p (l e) -> p l e", l=4))
            bk_sb = const.tile([128, NLIN], f32)
            nc.scalar.dma_start(bk_sb[:], bk[:])
            bq_sb = const.tile([128, 4], f32)
            nc.scalar.dma_start(bq_sb[:], bq[:])
            xkv_sb = const.tile([128, NSLOT, 2], bf16)
            nc.gpsimd.dma_start(xkv_sb[:], xkv[:].rearrange("p (c f) -> p c f", f=2))
            xt2b_sb = const.tile([128, max(NBSLOT, 1)], f32)
            nc.gpsimd.dma_start(xt2b_sb[:], xt2b[:])
            probe_sb = const.tile([128, 64], bf16)
            nc.gpsimd.dma_start(probe_sb[:], probe[:])
            step_sb = const.tile([NPAIR, NBLK], bf16)
            nc.gpsimd.dma_start(step_sb[:], stepm[:])

            xqT_sb = const.tile([128, NQ // 2], bf16)
            for nb in range(NQ // 2 // IBLK):
                sl = slice(nb * IBLK, (nb + 1) * IBLK)
                nc.scalar.dma_start(xqT_sb[:, sl], xqT[:, sl])
            t1b_sb = const.tile([CHUNK, NQ], f32)
            nc.scalar.dma_start(t1b_sb[:], t1p[:].partition_broadcast(CHUNK))
            xkT_sb = const.tile([128, KCOLS], bf16)
            nchk = -(-KCOLS // IBLK)
            for nb in range(nchk):
                sl = slice(nb * IBLK, min((nb + 1) * IBLK, KCOLS))
                nc.sync.dma_start(xkT_sb[:, sl], xkT[:, sl])

            out_sb = const.tile([2, NQ], f32)

            # ---- blocked K^T target: pair tiles with block-diagonal layout
            kTblk = const.tile([128, NSLOT, CHUNK], bf16)
            nc.gpsimd.memset(kTblk[0:64, :, 64:128], 0.0)
            nc.gpsimd.memset(kTblk[64:128, :, 0:64], 0.0)
            qT2 = const.tile([128, NQ], bf16)
            G_stack = const.tile([NPAIR, 2, 64], bf16)
            gstat = const.tile([128, NBLK, 2], bf16)
            nc.gpsimd.memset(gstat[:], 0.0)

            # ---- stacked MLPs (block-diagonal weights, both halves at once)
            def epilogue(dst, ps, bias, layer, eng):
                if eng == "act":
                    func = Relu if layer < NLIN - 1 else Identity
                    nc.scalar.activation(dst, ps, func, bias=bias)
                elif layer < NLIN - 1:
                    nc.vector.tensor_scalar(dst, ps, bias, 0.0, op0=add, op1=amax)
                else:
                    nc.vector.tensor_scalar(dst, ps, bias, None, op0=add)

            def mlp_hidden(cur, w_sb, b_sb, pool, nt, layer, eng):
                nxt = pool.tile([128, nt], bf16, tag="h")
                for nb in range(-(-nt // IBLK)):
                    sl = slice(nb * IBLK, min((nb + 1) * IBLK, nt))
                    csz = sl.stop - sl.start
                    ps = ps_a.tile([128, csz], f32, tag="a")
                    nc.tensor.matmul(
                        ps[:], w_sb[:, layer, :], cur[:, sl], start=True, stop=True
                    )
                    epilogue(nxt[:, sl], ps[:], b_sb[:, layer : layer + 1], layer, eng)
                return nxt

            hk, hq = xkT_sb, xqT_sb
            for layer in range(NLIN - 1):
                hk = mlp_hidden(hk, wk_sb, bk_sb, hkp, KCOLS, layer, "act")
                hq = mlp_hidden(hq, wq_sb, bq_sb, hqp, NQ // 2, layer, "dve")

            # final K layer: write straight into block-diagonal pair tiles.
            # Boundary chunks (slots >= NPAIR) first so the boundary S+mask
            # phase can start early; sorted chunks later, overlapping the
            # DVE mask drain.
            eng_flip = 0

            def k_final_chunk(nb):
                nonlocal eng_flip
                sl = slice(nb * IBLK, min((nb + 1) * IBLK, KCOLS))
                csz = sl.stop - sl.start
                ps = ps_a.tile([128, csz], f32, tag="a", name="ps")
                nc.tensor.matmul(
                    ps[:], wk_sb[:, NLIN - 1, :], hk[:, sl], start=True, stop=True
                )
                psv = ps[:].rearrange("p (a e) -> p a e", e=64)
                pair = slice(8 * nb, 8 * nb + csz // 64)
                bias = bk_sb[:, NLIN - 1 : NLIN]
                for half, csl in ((slice(0, 64), slice(0, 64)),
                                  (slice(64, 128), slice(64, 128))):
                    dst = kTblk[half, pair, csl]
                    srcv = psv[half, :, :]
                    if eng_flip % 2 == 0:
                        nc.scalar.activation(dst, srcv, Identity, bias=bias[half])
                    else:
                        nc.vector.tensor_scalar(dst, srcv, bias[half], None, op0=add)
                    eng_flip += 1

            nbchunk = NPAIR // 8  # first chunk holding boundary slots
            for nb in range(nbchunk, nchk):
                k_final_chunk(nb)

            # final Q layer: replicate Q^T onto both partition halves
            for nb in range(NQ // 2 // IBLK):
                sl = slice(nb * IBLK, (nb + 1) * IBLK)
                bias = bq_sb[:, NLIN - 1 : NLIN]
                for rep in range(2):
                    ps = ps_a.tile([128, IBLK], f32, tag="a")
                    nc.tensor.matmul(
                        ps[:], wq_sb[:, 2 + rep, :], hq[:, sl], start=True, stop=True
                    )
                    osl = slice(rep * (NQ // 2) + nb * IBLK,
                                rep * (NQ // 2) + (nb + 1) * IBLK)
                    epilogue(qT2[:, osl], ps[:], bias, NLIN - 1,
                             "act" if rep else "dve")

            # ---- boundary S + mask phase (early): masked S kept in SBUF so
            # the DVE mask work overlaps the probe/G phase on the PE.
            s_tiles = {}
            for b in range(NBLK):
                isl = slice(b * IBLK, (b + 1) * IBLK)
                for s in range(NBB[b]):
                    slot = boff[b] + s
                    bidx = slot - NPAIR
                    sp = ps_s.tile([CHUNK, IBLK], f32, tag="s", name="sp")
                    nc.tensor.matmul(
                        sp[:], kTblk[:, slot, :], qT2[:, isl],
                        start=True, stop=True, skip_group_check=True,
                    )
                    s_sb = spool.tile([CHUNK, IBLK], bf16, name="s_sb")
                    nc.vector.scalar_tensor_tensor(
                        s_sb[:], t1b_sb[:, isl], xt2b_sb[:, bidx:bidx + 1], sp[:],
                        op0=is_ge, op1=mult,
                    )
                    s_tiles[(b, s)] = s_sb

            # sorted K-final chunks (kTblk slots 0..NPAIR-1) for the G phase
            for nb in range(nbchunk):
                k_final_chunk(nb)

            # ---- G phase: G_j^T = V_j^T K_j via probe matmuls, 8 tiles/round
            # (probe-S rounds batched into one [128,512] PSUM tile + one ACT
            # copy; probe-AV rounds deferred one round so the copy overlaps
            # the next probe-S round on the PE)
            def emit_gav(rr_, sps_):
                gps = ps_og.tile([2, 512], f32, tag="og", name="gps")
                for slq in range(8):
                    j = rr_ * 8 + slq
                    nc.tensor.matmul(
                        gps[:, slq * 64:(slq + 1) * 64], xkv_sb[:, j, :],
                        sps_[:, slq * 64:(slq + 1) * 64],
                        start=True, stop=True, skip_group_check=True,
                    )
                gst = gstg.tile([2, 512], bf16, name="gst")
                nc.vector.tensor_copy(gst[:], gps[:])
                for c in range(2):
                    nc.gpsimd.dma_start(
                        G_stack[rr_ * 8:(rr_ + 1) * 8, c, :], gst[c:c + 1, :]
                    )

            prevg = None
            for r in range(NPAIR // 8):
                spb = ps_a.tile([128, 512], f32, tag="a", name="spb")
                for slq in range(8):
                    j = r * 8 + slq
                    nc.tensor.matmul(
                        spb[:, slq * 64:(slq + 1) * 64], kTblk[:, j, :],
                        probe_sb[:], start=True, stop=True, skip_group_check=True,
                    )
                sps = gpool.tile([128, 512], bf16, name="sps")
                nc.scalar.copy(sps[:], spb[:])
                if prevg is not None:
                    emit_gav(*prevg)
                prevg = (r, sps)
            emit_gav(*prevg)

            # ---- prefix select: G_pref_b = sum_{j < F[b]} G_j (step is data!)
            psel = ps_s.tile([64, 2 * NBLK], f32, tag="s")
            for c in range(2):
                nc.tensor.matmul(
                    psel[:, c * NBLK:(c + 1) * NBLK], G_stack[:, c, :], step_sb[:],
                    start=True, stop=True, skip_group_check=True,
                )
            for b in range(NBLK):
                for c in range(2):
                    i = c * NBLK + b
                    nc.scalar.copy(gstat[0:64, b, c:c + 1], psel[:, i:i + 1])

            # ---- output accumulation: full-region matmul + boundary AVs
            for b in range(NBLK):
                isl = slice(b * IBLK, (b + 1) * IBLK)
                ov = ps_og.tile([2, IBLK], f32, tag="og", name="ov")
                nc.tensor.matmul(
                    ov[:], gstat[:, b, :], qT2[:, isl],
                    start=True, stop=False, skip_group_check=True,
                )
                for s in range(NBB[b]):
                    slot = boff[b] + s
                    nc.tensor.matmul(
                        ov[:], xkv_sb[:, slot, :], s_tiles[(b, s)][:],
                        start=False, stop=(s == NBB[b] - 1),
                        skip_group_check=True,
                    )
                nc.scalar.copy(out_sb[:, isl], ov[:])

            nc.sync.dma_start(out[:], out_sb[:])

    nc.compile()
    return nc


def kernel(x1, x2, x3, x4, Wq_w, Wq_b, Wk_w, Wk_b):
    from concourse.bass_utils import run_bass_kernel_spmd

    global LAST_RESULTS

    xs = [np.asarray(a, dtype=np.float32)[0, 0] for a in (x1, x2, x3, x4)]
    Wq_w = np.asarray(Wq_w, dtype=np.float32)
    Wq_b = np.asarray(Wq_b, dtype=np.float32)
    Wk_w = np.asarray(Wk_w, dtype=np.float32)
    Wk_b = np.asarray(Wk_b, dtype=np.float32)

    t1 = xs[0][:, -1]
    t2s = [x[:, -1] for x in xs]

    # ---- per-core full/boundary classification (exact, from timestamps)
    FJ = {}  # (m, p) -> (F[b], J[b])
    NBB = [1] * NBLK
    for p in range(2):
        qoff = NQ * p
        for m in range(M):
            F, J = [], []
            for b in range(NBLK):
                lo = t1[qoff + b * IBLK]
                hi = t1[qoff + b * IBLK + IBLK - 1]
                nfull = int(np.searchsorted(t2s[m], lo, side="right"))
                nvis = int(np.searchsorted(t2s[m], hi, side="right"))
                F.append(nfull // CHUNK)
                J.append(-(-nvis // CHUNK))
                NBB[b] = max(NBB[b], J[b] - F[b])
            FJ[(m, p)] = (F, J)

    nc = _build_program(NBB)

    NBSLOT = sum(NBB)
    boff = [sum(NBB[:b]) for b in range(NBLK)]

    # ---- host packing
    def blockdiag(Wl):
        b = np.zeros((128, 128), np.float32)
        b[:64, :64] = Wl
        b[64:, 64:] = Wl
        return b

    # Q weights: layers 0,1 blockdiag; final as [[W,W],[0,0]] and [[0,0],[W,W]]
    wq_h = np.zeros((4, 128, 128), np.float32)
    for l in range(NLIN - 1):
        wq_h[l] = blockdiag(Wq_w[l])
    wq_h[2, :64, :64] = Wq_w[2]
    wq_h[2, :64, 64:] = Wq_w[2]
    wq_h[3, 64:, :64] = Wq_w[2]
    wq_h[3, 64:, 64:] = Wq_w[2]
    wq_h = np.ascontiguousarray(wq_h.transpose(1, 0, 2).reshape(128, 4 * 128))
    bq_h = np.tile(Wq_b.T, (2, 1))  # [128, 3]
    bq_h = np.ascontiguousarray(
        np.concatenate([bq_h, bq_h[:, 2:3]], axis=1)
    )  # [128, 4]

    probe_h = np.ascontiguousarray(
        np.concatenate([np.eye(64, dtype=np.float32)] * 2, axis=0)
    )  # [128, 64]

    x1T = np.ascontiguousarray(xs[0].T)

    def pack_tile(xrows):
        """[128, D] key rows -> ([128, 64] xkT block, [128, 2] V, [128] t2)."""
        ev, od = xrows[0:64], xrows[64:128]
        blk = np.concatenate([ev.T, od.T], axis=0)  # [128, 64]
        v = np.concatenate([ev[:, 0:2], od[:, 0:2]], axis=0)  # [128, 2]
        tt = np.concatenate([ev[:, -1], od[:, -1]], axis=0)  # [128]
        return blk, v, tt

    in_maps = []
    for c in range(8):
        m, p = c // 2, c % 2
        xm = xs[m]
        qoff = NQ * p
        F, J = FJ[(m, p)]

        NSLOT = NPAIR + NBSLOT
        xkT_h = np.zeros((128, NSLOT * 64), np.float32)
        xkv_h = np.zeros((128, NSLOT, 2), np.float32)
        xt2b_h = np.full((128, max(NBSLOT, 1)), 1e30, np.float32)
        for j in range(NPAIR):
            blk, v, tt = pack_tile(xm[CHUNK * j:CHUNK * (j + 1)])
            xkT_h[:, 64 * j:64 * (j + 1)] = blk
            xkv_h[:, j] = v
        for b in range(NBLK):
            for s in range(NBB[b]):
                t = F[b] + s
                slot = NPAIR + boff[b] + s
                if t < J[b]:
                    blk, v, tt = pack_tile(xm[CHUNK * t:CHUNK * (t + 1)])
                    xkT_h[:, 64 * slot:64 * (slot + 1)] = blk
                    xkv_h[:, slot] = v
                    xt2b_h[:, boff[b] + s] = tt
        step_h = np.zeros((NPAIR, NBLK), np.float32)
        for b in range(NBLK):
            step_h[: F[b], b] = 1.0

        wk_h = np.stack([blockdiag(Wk_w[m][l]) for l in range(NLIN)])
        wk_h = np.ascontiguousarray(wk_h.transpose(1, 0, 2).reshape(128, NLIN * 128))
        bk_h = np.ascontiguousarray(np.tile(Wk_b[m].T, (2, 1)))  # [128, 3]

        # query-side: contiguous half, [first 1024 | second 1024] stacking
        xq = x1T[:, qoff:qoff + NQ]  # [64, 2048]
        xqT_h = np.concatenate([xq[:, : NQ // 2], xq[:, NQ // 2:]], axis=0)

        in_maps.append(
            {
                "xqT": np.ascontiguousarray(xqT_h).astype(BF16),
                "xkT": xkT_h.astype(BF16),
                "xkv": np.ascontiguousarray(xkv_h.reshape(128, NSLOT * 2)).astype(BF16),
                "xt2b": xt2b_h,
                "t1p": np.ascontiguousarray(t1[qoff:qoff + NQ][None, :]),
                "probe": probe_h.astype(BF16),
                "stepm": step_h.astype(BF16),
                "wq": wq_h.astype(BF16),
                "bq": bq_h,
                "wk": wk_h.astype(BF16),
                "bk": bk_h,
            }
        )

    res = run_bass_kernel_spmd(nc, in_maps, core_ids=list(range(8)))
    LAST_RESULTS = res

    # ---- gather: sum over modalities per contiguous half, transpose
    acc = np.zeros((2, T), dtype=np.float32)
    for c in range(8):
        m, p = c // 2, c % 2
        acc[:, NQ * p:NQ * (p + 1)] += res.results[c]["out"]
    return np.ascontiguousarray(acc.T)[None]


# revision 9
# speedup vs baseline: 1.1280x; 1.0274x over previous
"""Trainium2 Bass kernel for masked multi-modal causal dot-product attention.

Computation (reference):
  Q = mlp(x1, Wq)               # (4096, 64), 3 linear layers, relu between
  for m in 0..3:
    K_m = mlp(x_m, Wk[m])       # (4096, 64)
    mask_m[i,j] = t2_m[j] <= t1[i]   (timestamps sorted -> staircase mask)
    acc += ((Q @ K_m.T) * mask_m) @ x_m[:, :2]
  out = acc  # (1, 4096, 2)

Sharding: 8 cores = 4 modalities x 2 contiguous query halves (2048 queries
each). One SPMD program; per-core variation lives in the input tensors.

Key algebraic optimization: for key tiles FULLY visible to a whole query
block, ((Q K^T) * 1) V = Q (K^T V). Per 128-key pair tile j we form
G_j^T = V_j^T K_j (2x64) with two tiny matmuls:
  - probe:  sp = kTblk_j^T @ [I64; I64]  -> K values, keys on partitions
  - reduce: G_j^T = xkv_j^T @ sp         -> PSUM (2, 64)
G tiles are staged to SBUF and DMA'd into a [32, 2, 64] partition-stacked
array; a step-vector matmul (host-built, per-core data) then selects the
prefix sum G_pref_b = sum_{j < F[b]} G_j for each query block -> the whole
fully-visible region costs ONE 512-col matmul per block. Only the ~5 boundary
tiles per block (keys whose timestamp falls inside the block's time span) run
the explicit S -> fused mask-multiply (DVE scalar_tensor_tensor) -> AV path.
Boundary keys are host-gathered into fixed tile slots so a single program
serves all cores; padded slots use t2=+inf and mask to zero.

Packing (from baseline): feature dim 64 is packed to contraction 128
everywhere (block-diagonal MLP weights on stacked halves, block-diagonal
K^T pair tiles, Q^T replicated onto both partition halves). All matmuls f32r.
"""

import os
import sys

import numpy as np
import ml_dtypes

BF16 = ml_dtypes.bfloat16

sys.path.insert(0, "/opt/trn_rl_repo")

T = 4096
D = 64
M = 4
NLIN = 3
NQ = 2048          # queries per core (contiguous half)
CHUNK = 128        # keys per pair tile (64 even + 64 odd)
NPAIR = T // CHUNK  # 32 sorted pair tiles
IBLK = 512         # query block (moving dim)
NBLK = NQ // IBLK  # 4 query blocks per core

LAST_RESULTS = None


def _build_program(NBB):
    """NBB[b]: boundary slots for query block b (same for all cores; per-core
    variation is in the gathered input data)."""
    import concourse.bacc as bacc
    import concourse.mybir as mybir
    import concourse.tile as tile

    f32 = mybir.dt.float32
    f32r = mybir.dt.float32r
    bf16 = mybir.dt.bfloat16
    Relu = mybir.ActivationFunctionType.Relu
    Identity = mybir.ActivationFunctionType.Identity
    is_ge = mybir.AluOpType.is_ge
    add = mybir.AluOpType.add
    amax = mybir.AluOpType.max
    mult = mybir.AluOpType.mult

    NBSLOT = sum(NBB)             # total boundary slots
    NSLOT = NPAIR + NBSLOT        # total pair tiles in kTblk
    KCOLS = NSLOT * 64            # K-MLP moving columns
    boff = [NPAIR + sum(NBB[:b]) for b in range(NBLK)]  # first slot of block b

    nc = bacc.Bacc("TRN2", target_bir_lowering=False, debug=False, num_devices=8)

    xqT = nc.dram_tensor("xqT", [128, NQ // 2], bf16, kind="ExternalInput")
    xkT = nc.dram_tensor("xkT", [128, KCOLS], bf16, kind="ExternalInput")
    xkv = nc.dram_tensor("xkv", [128, NSLOT * 2], bf16, kind="ExternalInput")
    xt2b = nc.dram_tensor("xt2b", [128, max(NBSLOT, 1)], f32, kind="ExternalInput")
    t1p = nc.dram_tensor("t1p", [1, NQ], f32, kind="ExternalInput")
    probe = nc.dram_tensor("probe", [128, 64], bf16, kind="ExternalInput")
    stepm = nc.dram_tensor("stepm", [NPAIR, NBLK], bf16, kind="ExternalInput")
    wq = nc.dram_tensor("wq", [128, 4 * 128], bf16, kind="ExternalInput")
    bq = nc.dram_tensor("bq", [128, 4], f32, kind="ExternalInput")
    wk = nc.dram_tensor("wk", [128, NLIN * 128], bf16, kind="ExternalInput")
    bk = nc.dram_tensor("bk", [128, NLIN], f32, kind="ExternalInput")
    out = nc.dram_tensor("out", [2, NQ], f32, kind="ExternalOutput")

    def rr(ap):
        return ap.bitcast(f32r)

    with tile.TileContext(nc) as tc:
        with (
            tc.tile_pool(name="const", bufs=1) as const,
            tc.tile_pool(name="hq", bufs=2) as hqp,
            tc.tile_pool(name="hk", bufs=2) as hkp,
            tc.tile_pool(name="spool", bufs=NBSLOT) as spool,
            tc.tile_pool(name="gpool", bufs=3) as gpool,
            tc.tile_pool(name="gstg", bufs=2) as gstg,
            tc.tile_pool(name="ps_a", bufs=3, space="PSUM") as ps_a,
            tc.tile_pool(name="ps_s", bufs=3, space="PSUM") as ps_s,
            tc.tile_pool(name="ps_og", bufs=2, space="PSUM") as ps_og,
        ):
            # ---- inputs -> SBUF (weights first, x chunked for overlap)
            wk_sb = const.tile([128, NLIN, 128], bf16)
            nc.scalar.dma_start(wk_sb[:], wk[:].rearrange("p (l e) -> p l e", l=NLIN))
            wq_sb = const.tile([128, 4, 128], bf16)
            nc.scalar.dma_start(wq_sb[:], wq[:].rearrange("p (l e) -> p l e", l=4))
            bk_sb = const.tile([128, NLIN], f32)
            nc.scalar.dma_start(bk_sb[:], bk[:])
            bq_sb = const.tile([128, 4], f32)
            nc.scalar.dma_start(bq_sb[:], bq[:])
            xkv_sb = const.tile([128, NSLOT, 2], bf16)
            nc.gpsimd.dma_start(xkv_sb[:], xkv[:].rearrange("p (c f) -> p c f", f=2))
            xt2b_sb = const.tile([128, max(NBSLOT, 1)], f32)
            nc.gpsimd.dma_start(xt2b_sb[:], xt2b[:])
            probe_sb = const.tile([128, 64], bf16)
            nc.gpsimd.dma_start(probe_sb[:], probe[:])
            step_sb = const.tile([NPAIR, NBLK], bf16)
            nc.gpsimd.dma_start(step_sb[:], stepm[:])

            xkT_sb = const.tile([128, KCOLS], bf16)
            nchk = -(-KCOLS // IBLK)
            for nb in range(nchk):
                sl = slice(nb * IBLK, min((nb + 1) * IBLK, KCOLS))
                eng = nc.sync if nb % 2 == 0 else nc.scalar
                eng.dma_start(xkT_sb[:, sl], xkT[:, sl])
            xqT_sb = const.tile([128, NQ // 2], bf16)
            for nb in range(NQ // 2 // IBLK):
                sl = slice(nb * IBLK, (nb + 1) * IBLK)
                nc.scalar.dma_start(xqT_sb[:, sl], xqT[:, sl])
            t1b_sb = const.tile([CHUNK, NQ], f32)
            nc.scalar.dma_start(t1b_sb[:], t1p[:].partition_broadcast(CHUNK))

            out_sb = const.tile([2, NQ], f32)

            # ---- blocked K^T target: pair tiles with block-diagonal layout
            kTblk = const.tile([128, NSLOT, CHUNK], bf16)
            nc.vector.memset(kTblk[0:64, :, 64:128], 0.0)
            nc.vector.memset(kTblk[64:128, :, 0:64], 0.0)
            qT2 = const.tile([128, NQ], bf16)
            G_stack = const.tile([NPAIR, 2, 64], bf16)
            gstat = const.tile([128, NBLK, 2], bf16)
            nc.gpsimd.memset(gstat[:], 0.0)

            # ---- stacked MLPs (block-diagonal weights, both halves at once)
            def epilogue(dst, ps, bias, layer, eng):
                if eng == "act":
                    func = Relu if layer < NLIN - 1 else Identity
                    nc.scalar.activation(dst, ps, func, bias=bias)
                elif layer < NLIN - 1:
                    nc.vector.tensor_scalar(dst, ps, bias, 0.0, op0=add, op1=amax)
                else:
                    nc.vector.tensor_scalar(dst, ps, bias, None, op0=add)

            def mlp_hidden(cur, w_sb, b_sb, pool, nt, layer, eng):
                nxt = pool.tile([128, nt], bf16, tag="h")
                for nb in range(-(-nt // IBLK)):
                    sl = slice(nb * IBLK, min((nb + 1) * IBLK, nt))
                    csz = sl.stop - sl.start
                    ps = ps_a.tile([128, csz], f32, tag="a")
                    nc.tensor.matmul(
                        ps[:], w_sb[:, layer, :], cur[:, sl], start=True, stop=True
                    )
                    epilogue(nxt[:, sl], ps[:], b_sb[:, layer : layer + 1], layer, eng)
                return nxt

            hk, hq = xkT_sb, xqT_sb
            for layer in range(NLIN - 1):
                hk = mlp_hidden(hk, wk_sb, bk_sb, hkp, KCOLS, layer, "act")
                hq = mlp_hidden(hq, wq_sb, bq_sb, hqp, NQ // 2, layer, "dve")

            # final K layer: write straight into block-diagonal pair tiles.
            # Boundary chunks (slots >= NPAIR) first so the boundary S+mask
            # phase can start early; sorted chunks later, overlapping the
            # DVE mask drain.
            eng_flip = 0

            def k_final_chunk(nb):
                nonlocal eng_flip
                sl = slice(nb * IBLK, min((nb + 1) * IBLK, KCOLS))
                csz = sl.stop - sl.start
                ps = ps_a.tile([128, csz], f32, tag="a", name="ps")
                nc.tensor.matmul(
                    ps[:], wk_sb[:, NLIN - 1, :], hk[:, sl], start=True, stop=True
                )
                psv = ps[:].rearrange("p (a e) -> p a e", e=64)
                pair = slice(8 * nb, 8 * nb + csz // 64)
                bias = bk_sb[:, NLIN - 1 : NLIN]
                for half, csl in ((slice(0, 64), slice(0, 64)),
                                  (slice(64, 128), slice(64, 128))):
                    dst = kTblk[half, pair, csl]
                    srcv = psv[half, :, :]
                    if eng_flip % 2 == 0:
                        nc.scalar.activation(dst, srcv, Identity, bias=bias[half])
                    else:
                        nc.vector.tensor_scalar(dst, srcv, bias[half], None, op0=add)
                    eng_flip += 1

            nbchunk = NPAIR // 8  # first chunk holding boundary slots
            for nb in range(nbchunk, nchk):
                k_final_chunk(nb)

            # final Q layer: replicate Q^T onto both partition halves
            for nb in range(NQ // 2 // IBLK):
                sl = slice(nb * IBLK, (nb + 1) * IBLK)
                bias = bq_sb[:, NLIN - 1 : NLIN]
                for rep in range(2):
                    ps = ps_a.tile([128, IBLK], f32, tag="a")
                    nc.tensor.matmul(
                        ps[:], wq_sb[:, 2 + rep, :], hq[:, sl], start=True, stop=True
                    )
                    osl = slice(rep * (NQ // 2) + nb * IBLK,
                                rep * (NQ // 2) + (nb + 1) * IBLK)
                    epilogue(qT2[:, osl], ps[:], bias, NLIN - 1,
                             "act" if rep else "dve")

            # ---- boundary S + mask phase (early): masked S kept in SBUF so
            # the DVE mask work overlaps the probe/G phase on the PE.
            s_tiles = {}
            for b in range(NBLK):
                isl = slice(b * IBLK, (b + 1) * IBLK)
                for s in range(NBB[b]):
                    slot = boff[b] + s
                    bidx = slot - NPAIR
                    sp = ps_s.tile([CHUNK, IBLK], f32, tag="s", name="sp")
                    nc.tensor.matmul(
                        sp[:], kTblk[:, slot, :], qT2[:, isl],
                        start=True, stop=True, skip_group_check=True,
                    )
                    s_sb = spool.tile([CHUNK, IBLK], bf16, name="s_sb")
                    nc.vector.scalar_tensor_tensor(
                        s_sb[:], t1b_sb[:, isl], xt2b_sb[:, bidx:bidx + 1], sp[:],
                        op0=is_ge, op1=mult,
                    )
                    s_tiles[(b, s)] = s_sb

            # sorted K-final chunks (kTblk slots 0..NPAIR-1) for the G phase
            for nb in range(nbchunk):
                k_final_chunk(nb)

            # ---- G phase: G_j^T = V_j^T K_j via probe matmuls, 8 tiles/round
            # (probe-S rounds batched into one [128,512] PSUM tile + one ACT
            # copy; probe-AV rounds deferred one round so the copy overlaps
            # the next probe-S round on the PE)
            def emit_gav(rr_, sps_):
                gps = ps_og.tile([2, 512], f32, tag="og", name="gps")
                for slq in range(8):
                    j = rr_ * 8 + slq
                    nc.tensor.matmul(
                        gps[:, slq * 64:(slq + 1) * 64], xkv_sb[:, j, :],
                        sps_[:, slq * 64:(slq + 1) * 64],
                        start=True, stop=True, skip_group_check=True,
                    )
                gst = gstg.tile([2, 512], bf16, name="gst")
                nc.vector.tensor_copy(gst[:], gps[:])
                for c in range(2):
                    nc.sync.dma_start(
                        G_stack[rr_ * 8:(rr_ + 1) * 8, c, :], gst[c:c + 1, :]
                    )

            prevg = None
            for r in range(NPAIR // 8):
                spb = ps_a.tile([128, 512], f32, tag="a", name="spb")
                for slq in range(8):
                    j = r * 8 + slq
                    nc.tensor.matmul(
                        spb[:, slq * 64:(slq + 1) * 64], kTblk[:, j, :],
                        probe_sb[:], start=True, stop=True, skip_group_check=True,
                    )
                sps = gpool.tile([128, 512], bf16, name="sps")
                nc.scalar.copy(sps[:], spb[:])
                if prevg is not None:
                    emit_gav(*prevg)
                prevg = (r, sps)
            emit_gav(*prevg)

            # ---- prefix select: G_pref_b = sum_{j < F[b]} G_j (step is data!)
            psel = ps_s.tile([64, 2 * NBLK], f32, tag="s")
            for c in range(2):
                nc.tensor.matmul(
                    psel[:, c * NBLK:(c + 1) * NBLK], G_stack[:, c, :], step_sb[:],
                    start=True, stop=True, skip_group_check=True,
                )
            for b in range(NBLK):
                for c in range(2):
                    i = c * NBLK + b
                    nc.scalar.copy(gstat[0:64, b, c:c + 1], psel[:, i:i + 1])

            # ---- output accumulation: full-region matmul + boundary AVs
            for b in range(NBLK):
                isl = slice(b * IBLK, (b + 1) * IBLK)
                ov = ps_og.tile([2, IBLK], f32, tag="og", name="ov")
                nc.tensor.matmul(
                    ov[:], gstat[:, b, :], qT2[:, isl],
                    start=True, stop=False, skip_group_check=True,
                )
                for s in range(NBB[b]):
                    slot = boff[b] + s
                    nc.tensor.matmul(
                        ov[:], xkv_sb[:, slot, :], s_tiles[(b, s)][:],
                        start=False, stop=(s == NBB[b] - 1),
                        skip_group_check=True,
                    )
                nc.scalar.copy(out_sb[:, isl], ov[:])
                nc.sync.dma_start(out[:, isl], out_sb[:, isl])

    nc.compile()
    return nc


def kernel(x1, x2, x3, x4, Wq_w, Wq_b, Wk_w, Wk_b):
    from concourse.bass_utils import run_bass_kernel_spmd

    global LAST_RESULTS

    xs = [np.asarray(a, dtype=np.float32)[0, 0] for a in (x1, x2, x3, x4)]
    Wq_w = np.asarray(Wq_w, dtype=np.float32)
    Wq_b = np.asarray(Wq_b, dtype=np.float32)
    Wk_w = np.asarray(Wk_w, dtype=np.float32)
    Wk_b = np.asarray(Wk_b, dtype=np.float32)

    t1 = xs[0][:, -1]
    t2s = [x[:, -1] for x in xs]

    # ---- per-core full/boundary classification (exact, from timestamps)
    FJ = {}  # (m, p) -> (F[b], J[b])
    NBB = [1] * NBLK
    for p in range(2):
        qoff = NQ * p
        for m in range(M):
            F, J = [], []
            for b in range(NBLK):
                lo = t1[qoff + b * IBLK]
                hi = t1[qoff + b * IBLK + IBLK - 1]
                nfull = int(np.searchsorted(t2s[m], lo, side="right"))
                nvis = int(np.searchsorted(t2s[m], hi, side="right"))
                F.append(nfull // CHUNK)
                J.append(-(-nvis // CHUNK))
                NBB[b] = max(NBB[b], J[b] - F[b])
            FJ[(m, p)] = (F, J)

    nc = _build_program(NBB)

    NBSLOT = sum(NBB)
    boff = [sum(NBB[:b]) for b in range(NBLK)]

    # ---- host packing
    def blockdiag(Wl):
        b = np.zeros((128, 128), np.float32)
        b[:64, :64] = Wl
        b[64:, 64:] = Wl
        return b

    # Q weights: layers 0,1 blockdiag; final as [[W,W],[0,0]] and [[0,0],[W,W]]
    wq_h = np.zeros((4, 128, 128), np.float32)
    for l in range(NLIN - 1):
        wq_h[l] = blockdiag(Wq_w[l])
    wq_h[2, :64, :64] = Wq_w[2]
    wq_h[2, :64, 64:] = Wq_w[2]
    wq_h[3, 64:, :64] = Wq_w[2]
    wq_h[3, 64:, 64:] = Wq_w[2]
    wq_h = np.ascontiguousarray(wq_h.transpose(1, 0, 2).reshape(128, 4 * 128))
    bq_h = np.tile(Wq_b.T, (2, 1))  # [128, 3]
    bq_h = np.ascontiguousarray(
        np.concatenate([bq_h, bq_h[:, 2:3]], axis=1)
    )  # [128, 4]

    probe_h = np.ascontiguousarray(
        np.concatenate([np.eye(64, dtype=np.float32)] * 2, axis=0)
    )  # [128, 64]

    x1T = np.ascontiguousarray(xs[0].T)

    def pack_tile(xrows):
        """[128, D] key rows -> ([128, 64] xkT block, [128, 2] V, [128] t2)."""
        ev, od = xrows[0:64], xrows[64:128]
        blk = np.concatenate([ev.T, od.T], axis=0)  # [128, 64]
        v = np.concatenate([ev[:, 0:2], od[:, 0:2]], axis=0)  # [128, 2]
        tt = np.concatenate([ev[:, -1], od[:, -1]], axis=0)  # [128]
        return blk, v, tt

    in_maps = []
    for c in range(8):
        m, p = c // 2, c % 2
        xm = xs[m]
        qoff = NQ * p
        F, J = FJ[(m, p)]

        NSLOT = NPAIR + NBSLOT
        xkT_h = np.zeros((128, NSLOT * 64), np.float32)
        xkv_h = np.zeros((128, NSLOT, 2), np.float32)
        xt2b_h = np.full((128, max(NBSLOT, 1)), 1e30, np.float32)
        for j in range(NPAIR):
            blk, v, tt = pack_tile(xm[CHUNK * j:CHUNK * (j + 1)])
            xkT_h[:, 64 * j:64 * (j + 1)] = blk
            xkv_h[:, j] = v
        for b in range(NBLK):
            for s in range(NBB[b]):
                t = F[b] + s
                slot = NPAIR + boff[b] + s
                if t < J[b]:
                    blk, v, tt = pack_tile(xm[CHUNK * t:CHUNK * (t + 1)])
                    xkT_h[:, 64 * slot:64 * (slot + 1)] = blk
                    xkv_h[:, slot] = v
                    xt2b_h[:, boff[b] + s] = tt
        step_h = np.zeros((NPAIR, NBLK), np.float32)
        for b in range(NBLK):
            step_h[: F[b], b] = 1.0

        wk_h = np.stack([blockdiag(Wk_w[m][l]) for l in range(NLIN)])
        wk_h = np.ascontiguousarray(wk_h.transpose(1, 0, 2).reshape(128, NLIN * 128))
        bk_h = np.ascontiguousarray(np.tile(Wk_b[m].T, (2, 1)))  # [128, 3]

        # query-side: contiguous half, [first 1024 | second 1024] stacking
        xq = x1T[:, qoff:qoff + NQ]  # [64, 2048]
        xqT_h = np.concatenate([xq[:, : NQ // 2], xq[:, NQ // 2:]], axis=0)

        in_maps.append(
            {
                "xqT": np.ascontiguousarray(xqT_h).astype(BF16),
                "xkT": xkT_h.astype(BF16),
                "xkv": np.ascontiguousarray(xkv_h.reshape(128, NSLOT * 2)).astype(BF16),
                "xt2b": xt2b_h,
                "t1p": np.ascontiguousarray(t1[qoff:qoff + NQ][None, :]),
                "probe": probe_h.astype(BF16),
                "stepm": step_h.astype(BF16),
                "wq": wq_h.astype(BF16),
                "bq": bq_h,
                "wk": wk_h.astype(BF16),
                "bk": bk_h,
            }
        )

    res = run_bass_kernel_spmd(nc, in_maps, core_ids=list(range(8)))
    LAST_RESULTS = res

    # ---- gather: sum over modalities per contiguous half, transpose
    acc = np.zeros((2, T), dtype=np.float32)
    for c in range(8):
        m, p = c // 2, c % 2
        acc[:, NQ * p:NQ * (p + 1)] += res.results[c]["out"]
    return np.ascontiguousarray(acc.T)[None]


# revision 15
# speedup vs baseline: 1.2112x; 1.0738x over previous
"""Trainium2 Bass kernel for masked multi-modal causal dot-product attention.

Computation (reference):
  Q = mlp(x1, Wq)               # (4096, 64), 3 linear layers, relu between
  for m in 0..3:
    K_m = mlp(x_m, Wk[m])       # (4096, 64)
    mask_m[i,j] = t2_m[j] <= t1[i]   (timestamps sorted -> staircase mask)
    acc += ((Q @ K_m.T) * mask_m) @ x_m[:, :2]
  out = acc  # (1, 4096, 2)

Sharding: 8 cores = 4 modalities x 2 contiguous query halves (2048 queries
each). One SPMD program; per-core variation lives in the input tensors.

Key algebraic optimization: for key tiles FULLY visible to a whole query
block, ((Q K^T) * 1) V = Q (K^T V). Per 128-key pair tile j we form
G_j^T = V_j^T K_j (2x64) with two tiny matmuls:
  - probe:  sp = kTblk_j^T @ [I64; I64]  -> K values, keys on partitions
  - reduce: G_j^T = xkv_j^T @ sp         -> PSUM (2, 64)
G tiles are staged to SBUF and DMA'd into a [32, 2, 64] partition-stacked
array; a step-vector matmul (host-built, per-core data) then selects the
prefix sum G_pref_b = sum_{j < F[b]} G_j for each query block -> the whole
fully-visible region costs ONE 512-col matmul per block. Only the ~5 boundary
tiles per block (keys whose timestamp falls inside the block's time span) run
the explicit S -> fused mask-multiply (DVE scalar_tensor_tensor) -> AV path.
Boundary keys are host-gathered into fixed tile slots so a single program
serves all cores; padded slots use t2=+inf and mask to zero.

Packing (from baseline): feature dim 64 is packed to contraction 128
everywhere (block-diagonal MLP weights on stacked halves, block-diagonal
K^T pair tiles, Q^T replicated onto both partition halves). All matmuls f32r.
"""

import os
import sys

import numpy as np
import ml_dtypes

BF16 = ml_dtypes.bfloat16

sys.path.insert(0, "/opt/trn_rl_repo")

T = 4096
D = 64
M = 4
NLIN = 3
NQ = 2048          # queries per core (contiguous half)
CHUNK = 128        # keys per pair tile (64 even + 64 odd)
NPAIR = T // CHUNK  # 32 sorted pair tiles
IBLK = 512         # query block (moving dim)
NBLK = NQ // IBLK  # 4 query blocks per core

LAST_RESULTS = None


def _build_program(NBB):
    """NBB[b]: boundary slots for query block b (same for all cores; per-core
    variation is in the gathered input data)."""
    import concourse.bacc as bacc
    import concourse.mybir as mybir
    import concourse.tile as tile

    f32 = mybir.dt.float32
    f32r = mybir.dt.float32r
    bf16 = mybir.dt.bfloat16
    Relu = mybir.ActivationFunctionType.Relu
    Identity = mybir.ActivationFunctionType.Identity
    is_ge = mybir.AluOpType.is_ge
    add = mybir.AluOpType.add
    amax = mybir.AluOpType.max
    mult = mybir.AluOpType.mult

    NBSLOT = sum(NBB)             # total boundary slots
    NSLOT = NPAIR + NBSLOT        # total pair tiles in kTblk
    KCOLS = NSLOT * 64            # K-MLP moving columns
    boff = [NPAIR + sum(NBB[:b]) for b in range(NBLK)]  # first slot of block b

    nc = bacc.Bacc("TRN2", target_bir_lowering=False, debug=False, num_devices=8)

    xqT = nc.dram_tensor("xqT", [128, NQ // 2], bf16, kind="ExternalInput")
    xkT = nc.dram_tensor("xkT", [128, KCOLS], bf16, kind="ExternalInput")
    xkvG = nc.dram_tensor("xkvG", [128, NPAIR * 2], bf16, kind="ExternalInput")
    xkvB = nc.dram_tensor("xkvB", [128, max(NBSLOT, 1) * 8], bf16, kind="ExternalInput")
    xt2b = nc.dram_tensor("xt2b", [128, max(NBSLOT, 1)], f32, kind="ExternalInput")
    t1p = nc.dram_tensor("t1p", [1, NQ], f32, kind="ExternalInput")
    probe = nc.dram_tensor("probe", [128, 64], bf16, kind="ExternalInput")
    stepm = nc.dram_tensor("stepm", [NPAIR, NBLK], bf16, kind="ExternalInput")
    wq = nc.dram_tensor("wq", [128, 4 * 128], bf16, kind="ExternalInput")
    bq = nc.dram_tensor("bq", [128, 4], f32, kind="ExternalInput")
    wk = nc.dram_tensor("wk", [128, NLIN * 128], bf16, kind="ExternalInput")
    bk = nc.dram_tensor("bk", [128, NLIN], f32, kind="ExternalInput")
    out = nc.dram_tensor("out", [2, NQ], f32, kind="ExternalOutput")

    def rr(ap):
        return ap.bitcast(f32r)

    with tile.TileContext(nc) as tc:
        with (
            tc.tile_pool(name="const", bufs=1) as const,
            tc.tile_pool(name="hq", bufs=2) as hqp,
            tc.tile_pool(name="hk", bufs=2) as hkp,
            tc.tile_pool(name="spool", bufs=3) as spool,
            tc.tile_pool(name="gpool", bufs=3) as gpool,
            tc.tile_pool(name="gstg", bufs=2) as gstg,
            tc.tile_pool(name="ps_a", bufs=3, space="PSUM") as ps_a,
            tc.tile_pool(name="ps_s", bufs=2, space="PSUM") as ps_s,
            tc.tile_pool(name="ps_g", bufs=2, space="PSUM") as ps_g,
            tc.tile_pool(name="ps_ov", bufs=1, space="PSUM") as ps_ov,
        ):
            # ---- inputs -> SBUF (weights first, x chunked for overlap)
            wk_sb = const.tile([128, NLIN, 128], bf16)
            nc.scalar.dma_start(wk_sb[:], wk[:].rearrange("p (l e) -> p l e", l=NLIN))
            wq_sb = const.tile([128, 4, 128], bf16)
            nc.scalar.dma_start(wq_sb[:], wq[:].rearrange("p (l e) -> p l e", l=4))
            bk_sb = const.tile([128, NLIN], f32)
            nc.scalar.dma_start(bk_sb[:], bk[:])
            bq_sb = const.tile([128, 4], f32)
            nc.scalar.dma_start(bq_sb[:], bq[:])
            xkvG_sb = const.tile([128, NPAIR, 2], bf16)
            nc.gpsimd.dma_start(xkvG_sb[:], xkvG[:].rearrange("p (c f) -> p c f", f=2))
            xkvB_sb = const.tile([128, max(NBSLOT, 1), 8], bf16)
            nc.gpsimd.dma_start(xkvB_sb[:], xkvB[:].rearrange("p (c f) -> p c f", f=8))
            xt2b_sb = const.tile([128, max(NBSLOT, 1)], f32)
            nc.gpsimd.dma_start(xt2b_sb[:], xt2b[:])
            probe_sb = const.tile([128, 64], bf16)
            nc.gpsimd.dma_start(probe_sb[:], probe[:])
            step_sb = const.tile([NPAIR, NBLK], bf16)
            nc.gpsimd.dma_start(step_sb[:], stepm[:])

            xkT_sb = const.tile([128, KCOLS], bf16)
            nchk = -(-KCOLS // IBLK)
            for nb in range(nchk):
                sl = slice(nb * IBLK, min((nb + 1) * IBLK, KCOLS))
                eng = nc.sync if nb % 2 == 0 else nc.scalar
                eng.dma_start(xkT_sb[:, sl], xkT[:, sl])
            xqT_sb = const.tile([128, NQ // 2], bf16)
            for nb in range(NQ // 2 // IBLK):
                sl = slice(nb * IBLK, (nb + 1) * IBLK)
                nc.scalar.dma_start(xqT_sb[:, sl], xqT[:, sl])
            t1b_sb = const.tile([CHUNK, NQ], f32)
            nc.scalar.dma_start(t1b_sb[:], t1p[:].partition_broadcast(CHUNK))

            out_sb = const.tile([8, IBLK], f32)

            # ---- blocked K^T target: pair tiles with block-diagonal layout
            kTblk = const.tile([128, NSLOT, CHUNK], bf16)
            nc.vector.memset(kTblk[0:64, :, 64:128], 0.0)
            nc.vector.memset(kTblk[64:128, :, 0:64], 0.0)
            qT2 = const.tile([128, NQ], bf16)
            G_stack = const.tile([NPAIR, 2, 64], bf16)
            gstat = const.tile([128, NBLK, 8], bf16)
            nc.gpsimd.memset(gstat[:], 0.0)

            # ---- stacked MLPs (block-diagonal weights, both halves at once)
            def epilogue(dst, ps, bias, layer, eng):
                if eng == "act":
                    func = Relu if layer < NLIN - 1 else Identity
                    nc.scalar.activation(dst, ps, func, bias=bias)
                elif layer < NLIN - 1:
                    nc.vector.tensor_scalar(dst, ps, bias, 0.0, op0=add, op1=amax)
                else:
                    nc.vector.tensor_scalar(dst, ps, bias, None, op0=add)

            def mlp_hidden(cur, w_sb, b_sb, pool, nt, layer, eng):
                nxt = pool.tile([128, nt], bf16, tag="h")
                for nb in range(-(-nt // IBLK)):
                    sl = slice(nb * IBLK, min((nb + 1) * IBLK, nt))
                    csz = sl.stop - sl.start
                    ps = ps_a.tile([128, csz], f32, tag="a")
                    nc.tensor.matmul(
                        ps[:], w_sb[:, layer, :], cur[:, sl], start=True, stop=True
                    )
                    epilogue(nxt[:, sl], ps[:], b_sb[:, layer : layer + 1], layer, eng)
                return nxt

            hk, hq = xkT_sb, xqT_sb
            for layer in range(NLIN - 1):
                hk = mlp_hidden(hk, wk_sb, bk_sb, hkp, KCOLS, layer, "act")
                hq = mlp_hidden(hq, wq_sb, bq_sb, hqp, NQ // 2, layer, "dve")

            # final K layer: write straight into block-diagonal pair tiles.
            # Boundary chunks first (feed the S/mask pipeline); sorted chunks
            # are injected later as filler jobs.
            eng_flip = 0

            def k_final_chunk(nb, act_only=False):
                nonlocal eng_flip
                sl = slice(nb * IBLK, min((nb + 1) * IBLK, KCOLS))
                csz = sl.stop - sl.start
                ps = ps_a.tile([128, csz], f32, tag="a", name="ps")
                nc.tensor.matmul(
                    ps[:], wk_sb[:, NLIN - 1, :], hk[:, sl], start=True, stop=True
                )
                psv = ps[:].rearrange("p (a e) -> p a e", e=64)
                pair = slice(8 * nb, 8 * nb + csz // 64)
                bias = bk_sb[:, NLIN - 1 : NLIN]
                for half, csl in ((slice(0, 64), slice(0, 64)),
                                  (slice(64, 128), slice(64, 128))):
                    dst = kTblk[half, pair, csl]
                    srcv = psv[half, :, :]
                    if act_only or eng_flip % 2 == 0:
                        nc.scalar.activation(dst, srcv, Identity, bias=bias[half])
                    else:
                        nc.vector.tensor_scalar(dst, srcv, bias[half], None, op0=add)
                    eng_flip += 1

            nbchunk = NPAIR // 8  # first chunk index holding boundary slots
            for nb in range(nbchunk, nchk):
                k_final_chunk(nb)

            # final Q layer: replicate Q^T onto both partition halves
            for nb in range(NQ // 2 // IBLK):
                sl = slice(nb * IBLK, (nb + 1) * IBLK)
                bias = bq_sb[:, NLIN - 1 : NLIN]
                for rep in range(2):
                    ps = ps_a.tile([128, IBLK], f32, tag="a")
                    nc.tensor.matmul(
                        ps[:], wq_sb[:, 2 + rep, :], hq[:, sl], start=True, stop=True
                    )
                    osl = slice(rep * (NQ // 2) + nb * IBLK,
                                rep * (NQ // 2) + (nb + 1) * IBLK)
                    epilogue(qT2[:, osl], ps[:], bias, NLIN - 1,
                             "act" if rep else "dve")

            # ---- single-bank output accumulator: rows 2b:2b+2 = block b
            ov8 = ps_ov.tile([8, IBLK], f32, tag="ov")

            # ---- filler jobs injected into the S/mask/AV interleave
            sps_tiles = {}

            def job_kfin(nb):
                k_final_chunk(nb, act_only=True)

            def job_probe_s(r):
                spb = ps_a.tile([128, 512], f32, tag="a", name="spb")
                for slq in range(8):
                    j = r * 8 + slq
                    nc.tensor.matmul(
                        spb[:, slq * 64:(slq + 1) * 64], kTblk[:, j, :],
                        probe_sb[:], start=True, stop=True, skip_group_check=True,
                    )
                sps = gpool.tile([128, 512], bf16, name="sps")
                nc.scalar.copy(sps[:], spb[:])
                sps_tiles[r] = sps

            def job_probe_av(r):
                gps = ps_g.tile([2, 512], f32, tag="g", name="gps")
                sps = sps_tiles[r]
                for slq in range(8):
                    j = r * 8 + slq
                    nc.tensor.matmul(
                        gps[:, slq * 64:(slq + 1) * 64], xkvG_sb[:, j, :],
                        sps[:, slq * 64:(slq + 1) * 64],
                        start=True, stop=True, skip_group_check=True,
                    )
                gst = gstg.tile([2, 512], bf16, name="gst")
                nc.scalar.copy(gst[:], gps[:])
                for c in range(2):
                    nc.sync.dma_start(
                        G_stack[r * 8:(r + 1) * 8, c, :], gst[c:c + 1, :]
                    )

            def job_select():
                psel = ps_g.tile([64, 2 * NBLK], f32, tag="g")
                for c in range(2):
                    nc.tensor.matmul(
                        psel[:, c * NBLK:(c + 1) * NBLK], G_stack[:, c, :],
                        step_sb[:], start=True, stop=True, skip_group_check=True,
                    )
                for b in range(NBLK):
                    for c in range(2):
                        i = c * NBLK + b
                        nc.scalar.copy(gstat[0:64, b, 2 * b + c:2 * b + c + 1],
                                       psel[:, i:i + 1])

            def job_full(b):
                isl = slice(b * IBLK, (b + 1) * IBLK)
                nc.tensor.matmul(
                    ov8[:], gstat[:, b, :], qT2[:, isl],
                    start=False, stop=False, skip_group_check=True,
                )

            jobs = [lambda nb=nb: job_kfin(nb) for nb in range(nbchunk)]
            jobs += [lambda: job_probe_s(0)]
            for r in range(1, NPAIR // 8):
                jobs += [lambda r=r: (job_probe_s(r), job_probe_av(r - 1))]
            jobs += [lambda: job_probe_av(NPAIR // 8 - 1)]
            jobs += [job_select]
            jobs += [lambda b=b: job_full(b) for b in range(NBLK)]

            # ---- main interleave: S -> fused mask (DVE) -> AV, one job/iter
            def emit_av(b, s, first, last):
                slot = boff[b] + s
                bidx = slot - NPAIR
                nc.tensor.matmul(
                    ov8[:], xkvB_sb[:, bidx, :], s_tiles[(b, s)][:],
                    start=first, stop=last, skip_group_check=True,
                )

            s_tiles = {}
            seq = [(b, s) for b in range(NBLK) for s in range(NBB[b])]
            it = 0
            prev_av = None
            for b, s in seq:
                slot = boff[b] + s
                bidx = slot - NPAIR
                isl = slice(b * IBLK, (b + 1) * IBLK)
                if it < len(jobs):
                    jobs[it]()
                sp = ps_s.tile([CHUNK, IBLK], f32, tag="s", name="sp")
                nc.tensor.matmul(
                    sp[:], kTblk[:, slot, :], qT2[:, isl],
                    start=True, stop=True, skip_group_check=True,
                )
                s_sb = spool.tile([CHUNK, IBLK], bf16, name="s_sb")
                nc.vector.scalar_tensor_tensor(
                    s_sb[:], t1b_sb[:, isl], xt2b_sb[:, bidx:bidx + 1], sp[:],
                    op0=is_ge, op1=mult,
                )
                s_tiles[(b, s)] = s_sb
                if prev_av is not None:
                    emit_av(*prev_av, prev_av == seq[0], False)
                prev_av = (b, s)
                it += 1
            for j in range(it, len(jobs)):
                jobs[j]()
            emit_av(*prev_av, False, True)

            # ---- readout
            nc.scalar.copy(out_sb[:], ov8[:])
            for b in range(NBLK):
                isl = slice(b * IBLK, (b + 1) * IBLK)
                nc.sync.dma_start(out[:, isl], out_sb[2 * b:2 * b + 2, :])

    nc.compile()
    return nc


def kernel(x1, x2, x3, x4, Wq_w, Wq_b, Wk_w, Wk_b):
    from concourse.bass_utils import run_bass_kernel_spmd

    global LAST_RESULTS

    xs = [np.asarray(a, dtype=np.float32)[0, 0] for a in (x1, x2, x3, x4)]
    Wq_w = np.asarray(Wq_w, dtype=np.float32)
    Wq_b = np.asarray(Wq_b, dtype=np.float32)
    Wk_w = np.asarray(Wk_w, dtype=np.float32)
    Wk_b = np.asarray(Wk_b, dtype=np.float32)

    t1 = xs[0][:, -1]
    t2s = [x[:, -1] for x in xs]

    # ---- per-core full/boundary classification (exact, from timestamps)
    FJ = {}  # (m, p) -> (F[b], J[b])
    NBB = [1] * NBLK
    for p in range(2):
        qoff = NQ * p
        for m in range(M):
            F, J = [], []
            for b in range(NBLK):
                lo = t1[qoff + b * IBLK]
                hi = t1[qoff + b * IBLK + IBLK - 1]
                nfull = int(np.searchsorted(t2s[m], lo, side="right"))
                nvis = int(np.searchsorted(t2s[m], hi, side="right"))
                F.append(nfull // CHUNK)
                J.append(-(-nvis // CHUNK))
                NBB[b] = max(NBB[b], J[b] - F[b])
            FJ[(m, p)] = (F, J)

    nc = _build_program(NBB)

    NBSLOT = sum(NBB)
    boff = [sum(NBB[:b]) for b in range(NBLK)]

    # ---- host packing
    def blockdiag(Wl):
        b = np.zeros((128, 128), np.float32)
        b[:64, :64] = Wl
        b[64:, 64:] = Wl
        return b

    # Q weights: layers 0,1 blockdiag; final as [[W,W],[0,0]] and [[0,0],[W,W]]
    wq_h = np.zeros((4, 128, 128), np.float32)
    for l in range(NLIN - 1):
        wq_h[l] = blockdiag(Wq_w[l])
    wq_h[2, :64, :64] = Wq_w[2]
    wq_h[2, :64, 64:] = Wq_w[2]
    wq_h[3, 64:, :64] = Wq_w[2]
    wq_h[3, 64:, 64:] = Wq_w[2]
    wq_h = np.ascontiguousarray(wq_h.transpose(1, 0, 2).reshape(128, 4 * 128))
    bq_h = np.tile(Wq_b.T, (2, 1))  # [128, 3]
    bq_h = np.ascontiguousarray(
        np.concatenate([bq_h, bq_h[:, 2:3]], axis=1)
    )  # [128, 4]

    probe_h = np.ascontiguousarray(
        np.concatenate([np.eye(64, dtype=np.float32)] * 2, axis=0)
    )  # [128, 64]

    x1T = np.ascontiguousarray(xs[0].T)

    def pack_tile(xrows):
        """[128, D] key rows -> ([128, 64] xkT block, [128, 2] V, [128] t2)."""
        ev, od = xrows[0:64], xrows[64:128]
        blk = np.concatenate([ev.T, od.T], axis=0)  # [128, 64]
        v = np.concatenate([ev[:, 0:2], od[:, 0:2]], axis=0)  # [128, 2]
        tt = np.concatenate([ev[:, -1], od[:, -1]], axis=0)  # [128]
        return blk, v, tt

    in_maps = []
    for c in range(8):
        m, p = c // 2, c % 2
        xm = xs[m]
        qoff = NQ * p
        F, J = FJ[(m, p)]

        NSLOT = NPAIR + NBSLOT
        xkT_h = np.zeros((128, NSLOT * 64), np.float32)
        xkvG_h = np.zeros((128, NPAIR, 2), np.float32)
        xkvB_h = np.zeros((128, max(NBSLOT, 1), 8), np.float32)
        xt2b_h = np.full((128, max(NBSLOT, 1)), 1e30, np.float32)
        for j in range(NPAIR):
            blk, v, tt = pack_tile(xm[CHUNK * j:CHUNK * (j + 1)])
            xkT_h[:, 64 * j:64 * (j + 1)] = blk
            xkvG_h[:, j] = v
        for b in range(NBLK):
            for s in range(NBB[b]):
                t = F[b] + s
                slot = NPAIR + boff[b] + s
                if t < J[b]:
                    blk, v, tt = pack_tile(xm[CHUNK * t:CHUNK * (t + 1)])
                    xkT_h[:, 64 * slot:64 * (slot + 1)] = blk
                    xkvB_h[:, boff[b] + s, 2 * b:2 * b + 2] = v
                    xt2b_h[:, boff[b] + s] = tt
        step_h = np.zeros((NPAIR, NBLK), np.float32)
        if not os.environ.get('DBG_NO_FULL'):
            for b in range(NBLK):
                step_h[: F[b], b] = 1.0

        wk_h = np.stack([blockdiag(Wk_w[m][l]) for l in range(NLIN)])
        wk_h = np.ascontiguousarray(wk_h.transpose(1, 0, 2).reshape(128, NLIN * 128))
        bk_h = np.ascontiguousarray(np.tile(Wk_b[m].T, (2, 1)))  # [128, 3]

        # query-side: contiguous half, [first 1024 | second 1024] stacking
        xq = x1T[:, qoff:qoff + NQ]  # [64, 2048]
        xqT_h = np.concatenate([xq[:, : NQ // 2], xq[:, NQ // 2:]], axis=0)

        in_maps.append(
            {
                "xqT": np.ascontiguousarray(xqT_h).astype(BF16),
                "xkT": xkT_h.astype(BF16),
                "xkvG": np.ascontiguousarray(xkvG_h.reshape(128, NPAIR * 2)).astype(BF16),
                "xkvB": np.ascontiguousarray(
                    xkvB_h.reshape(128, max(NBSLOT, 1) * 8)).astype(BF16),
                "xt2b": xt2b_h,
                "t1p": np.ascontiguousarray(t1[qoff:qoff + NQ][None, :]),
                "probe": probe_h.astype(BF16),
                "stepm": step_h.astype(BF16),
                "wq": wq_h.astype(BF16),
                "bq": bq_h,
                "wk": wk_h.astype(BF16),
                "bk": bk_h,
            }
        )

    res = run_bass_kernel_spmd(nc, in_maps, core_ids=list(range(8)))
    LAST_RESULTS = res

    # ---- gather: sum over modalities per contiguous half, transpose
    acc = np.zeros((2, T), dtype=np.float32)
    for c in range(8):
        m, p = c // 2, c % 2
        acc[:, NQ * p:NQ * (p + 1)] += res.results[c]["out"]
    return np.ascontiguousarray(acc.T)[None]


# revision 16
# speedup vs baseline: 1.3526x; 1.1167x over previous
"""Trainium2 Bass kernel for masked multi-modal causal dot-product attention.

Computation (reference):
  Q = mlp(x1, Wq)               # (4096, 64), 3 linear layers, relu between
  for m in 0..3:
    K_m = mlp(x_m, Wk[m])       # (4096, 64)
    mask_m[i,j] = t2_m[j] <= t1[i]   (timestamps sorted -> staircase mask)
    acc += ((Q @ K_m.T) * mask_m) @ x_m[:, :2]
  out = acc  # (1, 4096, 2)

Sharding: 8 cores = 4 modalities x 2 contiguous query halves (2048 queries
each). One SPMD program; per-core variation lives in the input tensors.

Key algebraic optimization: for key tiles FULLY visible to a whole query
block, ((Q K^T) * 1) V = Q (K^T V). Per 128-key pair tile j we form
G_j^T = V_j^T K_j (2x64) with two tiny matmuls:
  - probe:  sp = kTblk_j^T @ [I64; I64]  -> K values, keys on partitions
  - reduce: G_j^T = xkv_j^T @ sp         -> PSUM (2, 64)
G tiles are staged to SBUF and DMA'd into a [32, 2, 64] partition-stacked
array; a step-vector matmul (host-built, per-core data) then selects the
prefix sum G_pref_b = sum_{j < F[b]} G_j for each query block -> the whole
fully-visible region costs ONE 512-col matmul per block. Only the ~5 boundary
tiles per block (keys whose timestamp falls inside the block's time span) run
the explicit S -> fused mask-multiply (DVE scalar_tensor_tensor) -> AV path.
Boundary keys are host-gathered into fixed tile slots so a single program
serves all cores; padded slots use t2=+inf and mask to zero.

Packing (from baseline): feature dim 64 is packed to contraction 128
everywhere (block-diagonal MLP weights on stacked halves, block-diagonal
K^T pair tiles, Q^T replicated onto both partition halves). All matmuls f32r.
"""

import os
import sys

import numpy as np
import ml_dtypes

BF16 = ml_dtypes.bfloat16

sys.path.insert(0, "/opt/trn_rl_repo")

T = 4096
D = 64
M = 4
NLIN = 3
NQ = 2048          # queries per core (contiguous half)
CHUNK = 128        # keys per pair tile (64 even + 64 odd)
NPAIR = T // CHUNK  # 32 sorted pair tiles
IBLK = 512         # query block (moving dim)
NBLK = NQ // IBLK  # 4 query blocks per core

LAST_RESULTS = None


def _build_program(NBB):
    """NBB[b]: boundary slots for query block b (same for all cores; per-core
    variation is in the gathered input data)."""
    import concourse.bacc as bacc
    import concourse.mybir as mybir
    import concourse.tile as tile

    f32 = mybir.dt.float32
    f32r = mybir.dt.float32r
    bf16 = mybir.dt.bfloat16
    Relu = mybir.ActivationFunctionType.Relu
    Identity = mybir.ActivationFunctionType.Identity
    is_ge = mybir.AluOpType.is_ge
    add = mybir.AluOpType.add
    amax = mybir.AluOpType.max
    mult = mybir.AluOpType.mult

    NBSLOT = sum(NBB)             # total boundary slots
    NSLOT = NPAIR + NBSLOT        # total pair tiles in kTblk
    KCOLS = NSLOT * 64            # K-MLP moving columns
    boff = [NPAIR + sum(NBB[:b]) for b in range(NBLK)]  # first slot of block b

    nc = bacc.Bacc("TRN2", target_bir_lowering=False, debug=False, num_devices=8)

    xqT = nc.dram_tensor("xqT", [128, NQ // 2], bf16, kind="ExternalInput")
    xkT = nc.dram_tensor("xkT", [128, KCOLS], bf16, kind="ExternalInput")
    xkvG = nc.dram_tensor("xkvG", [128, NPAIR * 2], bf16, kind="ExternalInput")
    xkvB = nc.dram_tensor("xkvB", [128, max(NBSLOT, 1) * 8], bf16, kind="ExternalInput")
    xt2b = nc.dram_tensor("xt2b", [128, max(NBSLOT, 1)], f32, kind="ExternalInput")
    t1p = nc.dram_tensor("t1p", [1, NQ], f32, kind="ExternalInput")
    probe = nc.dram_tensor("probe", [128, 64], bf16, kind="ExternalInput")
    stepm = nc.dram_tensor("stepm", [NPAIR, NBLK], bf16, kind="ExternalInput")
    wq = nc.dram_tensor("wq", [128, 4 * 128], bf16, kind="ExternalInput")
    bq = nc.dram_tensor("bq", [128, 4], f32, kind="ExternalInput")
    wk = nc.dram_tensor("wk", [128, NLIN * 128], bf16, kind="ExternalInput")
    bk = nc.dram_tensor("bk", [128, NLIN], f32, kind="ExternalInput")
    out = nc.dram_tensor("out", [2, NQ], f32, kind="ExternalOutput")

    def rr(ap):
        return ap.bitcast(f32r)

    with tile.TileContext(nc) as tc:
        with (
            tc.tile_pool(name="const", bufs=1) as const,
            tc.tile_pool(name="hq", bufs=2) as hqp,
            tc.tile_pool(name="hk", bufs=2) as hkp,
            tc.tile_pool(name="spool", bufs=3) as spool,
            tc.tile_pool(name="gpool", bufs=3) as gpool,
            tc.tile_pool(name="gstg", bufs=2) as gstg,
            tc.tile_pool(name="ps_a", bufs=3, space="PSUM") as ps_a,
            tc.tile_pool(name="ps_s", bufs=2, space="PSUM") as ps_s,
            tc.tile_pool(name="ps_g", bufs=2, space="PSUM") as ps_g,
            tc.tile_pool(name="ps_ov", bufs=1, space="PSUM") as ps_ov,
        ):
            # ---- inputs -> SBUF (weights first, x chunked for overlap)
            wk_sb = const.tile([128, NLIN, 128], bf16)
            nc.scalar.dma_start(wk_sb[:], wk[:].rearrange("p (l e) -> p l e", l=NLIN))
            bk_sb = const.tile([128, NLIN], f32)
            nc.scalar.dma_start(bk_sb[:], bk[:])
            xkvG_sb = const.tile([128, NPAIR, 2], bf16)
            nc.gpsimd.dma_start(xkvG_sb[:], xkvG[:].rearrange("p (c f) -> p c f", f=2))
            xkvB_sb = const.tile([128, max(NBSLOT, 1), 8], bf16)
            nc.gpsimd.dma_start(xkvB_sb[:], xkvB[:].rearrange("p (c f) -> p c f", f=8))
            xt2b_sb = const.tile([128, max(NBSLOT, 1)], f32)
            nc.gpsimd.dma_start(xt2b_sb[:], xt2b[:])
            probe_sb = const.tile([128, 64], bf16)
            nc.gpsimd.dma_start(probe_sb[:], probe[:])
            step_sb = const.tile([NPAIR, NBLK], bf16)
            nc.gpsimd.dma_start(step_sb[:], stepm[:])

            xkT_sb = const.tile([128, KCOLS], bf16)
            nchk = -(-KCOLS // IBLK)
            for nb in range(nchk):
                sl = slice(nb * IBLK, min((nb + 1) * IBLK, KCOLS))
                eng = nc.sync if nb % 2 == 0 else nc.scalar
                eng.dma_start(xkT_sb[:, sl], xkT[:, sl])
            wq_sb = const.tile([128, 4, 128], bf16)
            nc.scalar.dma_start(wq_sb[:], wq[:].rearrange("p (l e) -> p l e", l=4))
            bq_sb = const.tile([128, 4], f32)
            nc.scalar.dma_start(bq_sb[:], bq[:])
            xqT_sb = const.tile([128, NQ // 2], bf16)
            for nb in range(NQ // 2 // IBLK):
                sl = slice(nb * IBLK, (nb + 1) * IBLK)
                nc.scalar.dma_start(xqT_sb[:, sl], xqT[:, sl])
            t1b_sb = const.tile([CHUNK, NQ], f32)
            nc.scalar.dma_start(t1b_sb[:], t1p[:].partition_broadcast(CHUNK))

            out_sb = const.tile([8, IBLK], f32)

            # ---- blocked K^T target: pair tiles with block-diagonal layout
            kTblk = const.tile([128, NSLOT, CHUNK], bf16)
            nc.vector.memset(kTblk[0:64, :, 64:128], 0.0)
            nc.vector.memset(kTblk[64:128, :, 0:64], 0.0)
            qT2 = const.tile([128, NQ], bf16)
            G_stack = const.tile([NPAIR, 2, 64], bf16)
            gstat = const.tile([128, NBLK, 8], bf16)
            nc.gpsimd.memset(gstat[:], 0.0)

            # ---- stacked MLPs (block-diagonal weights, both halves at once)
            def epilogue(dst, ps, bias, layer, eng):
                if eng == "act":
                    func = Relu if layer < NLIN - 1 else Identity
                    nc.scalar.activation(dst, ps, func, bias=bias)
                elif layer < NLIN - 1:
                    nc.vector.tensor_scalar(dst, ps, bias, 0.0, op0=add, op1=amax)
                else:
                    nc.vector.tensor_scalar(dst, ps, bias, None, op0=add)

            def mlp_hidden(cur, w_sb, b_sb, pool, nt, layer, eng):
                nxt = pool.tile([128, nt], bf16, tag="h")
                for nb in range(-(-nt // IBLK)):
                    sl = slice(nb * IBLK, min((nb + 1) * IBLK, nt))
                    csz = sl.stop - sl.start
                    ps = ps_a.tile([128, csz], f32, tag="a")
                    nc.tensor.matmul(
                        ps[:], w_sb[:, layer, :], cur[:, sl], start=True, stop=True
                    )
                    epilogue(nxt[:, sl], ps[:], b_sb[:, layer : layer + 1], layer, eng)
                return nxt

            hk, hq = xkT_sb, xqT_sb
            for layer in range(NLIN - 1):
                hk = mlp_hidden(hk, wk_sb, bk_sb, hkp, KCOLS, layer, "act")
                hq = mlp_hidden(hq, wq_sb, bq_sb, hqp, NQ // 2, layer, "dve")

            # final K layer: write straight into block-diagonal pair tiles.
            # Boundary chunks first (feed the S/mask pipeline); sorted chunks
            # are injected later as filler jobs.
            eng_flip = 0

            def k_final_chunk(nb, act_only=False):
                nonlocal eng_flip
                sl = slice(nb * IBLK, min((nb + 1) * IBLK, KCOLS))
                csz = sl.stop - sl.start
                ps = ps_a.tile([128, csz], f32, tag="a", name="ps")
                nc.tensor.matmul(
                    ps[:], wk_sb[:, NLIN - 1, :], hk[:, sl], start=True, stop=True
                )
                psv = ps[:].rearrange("p (a e) -> p a e", e=64)
                pair = slice(8 * nb, 8 * nb + csz // 64)
                bias = bk_sb[:, NLIN - 1 : NLIN]
                for half, csl in ((slice(0, 64), slice(0, 64)),
                                  (slice(64, 128), slice(64, 128))):
                    dst = kTblk[half, pair, csl]
                    srcv = psv[half, :, :]
                    if act_only or eng_flip % 2 == 0:
                        nc.scalar.activation(dst, srcv, Identity, bias=bias[half])
                    else:
                        nc.vector.tensor_scalar(dst, srcv, bias[half], None, op0=add)
                    eng_flip += 1

            nbchunk = NPAIR // 8  # first chunk index holding boundary slots
            for nb in range(nbchunk, nchk):
                k_final_chunk(nb)

            # final Q layer: replicate Q^T onto both partition halves
            for nb in range(NQ // 2 // IBLK):
                sl = slice(nb * IBLK, (nb + 1) * IBLK)
                bias = bq_sb[:, NLIN - 1 : NLIN]
                for rep in range(2):
                    ps = ps_a.tile([128, IBLK], f32, tag="a")
                    nc.tensor.matmul(
                        ps[:], wq_sb[:, 2 + rep, :], hq[:, sl], start=True, stop=True
                    )
                    osl = slice(rep * (NQ // 2) + nb * IBLK,
                                rep * (NQ // 2) + (nb + 1) * IBLK)
                    epilogue(qT2[:, osl], ps[:], bias, NLIN - 1,
                             "act" if rep else "dve")

            # ---- single-bank output accumulator: rows 2b:2b+2 = block b
            ov8 = ps_ov.tile([8, IBLK], f32, tag="ov")

            # ---- filler jobs injected into the S/mask/AV interleave
            sps_tiles = {}

            def job_kfin(nb):
                k_final_chunk(nb, act_only=True)

            def job_probe_s(r):
                spb = ps_a.tile([128, 512], f32, tag="a", name="spb")
                for slq in range(8):
                    j = r * 8 + slq
                    nc.tensor.matmul(
                        spb[:, slq * 64:(slq + 1) * 64], kTblk[:, j, :],
                        probe_sb[:], start=True, stop=True, skip_group_check=True,
                    )
                sps = gpool.tile([128, 512], bf16, name="sps")
                nc.scalar.copy(sps[:], spb[:])
                sps_tiles[r] = sps

            def job_probe_av(r):
                gps = ps_g.tile([2, 512], f32, tag="g", name="gps")
                sps = sps_tiles[r]
                for slq in range(8):
                    j = r * 8 + slq
                    nc.tensor.matmul(
                        gps[:, slq * 64:(slq + 1) * 64], xkvG_sb[:, j, :],
                        sps[:, slq * 64:(slq + 1) * 64],
                        start=True, stop=True, skip_group_check=True,
                    )
                gst = gstg.tile([2, 512], bf16, name="gst")
                nc.scalar.copy(gst[:], gps[:])
                for c in range(2):
                    nc.sync.dma_start(
                        G_stack[r * 8:(r + 1) * 8, c, :], gst[c:c + 1, :]
                    )

            def job_select():
                psel = ps_g.tile([64, 2 * NBLK], f32, tag="g")
                for c in range(2):
                    nc.tensor.matmul(
                        psel[:, c * NBLK:(c + 1) * NBLK], G_stack[:, c, :],
                        step_sb[:], start=True, stop=True, skip_group_check=True,
                    )
                for b in range(NBLK):
                    for c in range(2):
                        i = c * NBLK + b
                        nc.scalar.copy(gstat[0:64, b, 2 * b + c:2 * b + c + 1],
                                       psel[:, i:i + 1])

            def job_full(b):
                isl = slice(b * IBLK, (b + 1) * IBLK)
                nc.tensor.matmul(
                    ov8[:], gstat[:, b, :], qT2[:, isl],
                    start=False, stop=False, skip_group_check=True,
                )

            jobs = [lambda nb=nb: job_kfin(nb) for nb in range(nbchunk)]
            jobs += [lambda: job_probe_s(0)]
            for r in range(1, NPAIR // 8):
                jobs += [lambda r=r: (job_probe_s(r), job_probe_av(r - 1))]
            jobs += [lambda: job_probe_av(NPAIR // 8 - 1)]

            # ---- main interleave: S -> fused mask (DVE) -> AV, one job/iter
            def emit_av(b, s, first, last):
                slot = boff[b] + s
                bidx = slot - NPAIR
                nc.tensor.matmul(
                    ov8[:], xkvB_sb[:, bidx, :], s_tiles[(b, s)][:],
                    start=first, stop=last, skip_group_check=True,
                )

            s_tiles = {}
            seq = [(b, s) for b in range(NBLK) for s in range(NBB[b])]
            it = 0
            prev_av = None
            for b, s in seq:
                slot = boff[b] + s
                bidx = slot - NPAIR
                isl = slice(b * IBLK, (b + 1) * IBLK)
                if it < len(jobs):
                    jobs[it]()
                sp = ps_s.tile([CHUNK, IBLK], f32, tag="s", name="sp")
                nc.tensor.matmul(
                    sp[:], kTblk[:, slot, :], qT2[:, isl],
                    start=True, stop=True, skip_group_check=True,
                )
                s_sb = spool.tile([CHUNK, IBLK], bf16, name="s_sb")
                nc.vector.scalar_tensor_tensor(
                    s_sb[:], t1b_sb[:, isl], xt2b_sb[:, bidx:bidx + 1], sp[:],
                    op0=is_ge, op1=mult,
                )
                s_tiles[(b, s)] = s_sb
                if prev_av is not None:
                    emit_av(*prev_av, prev_av == seq[0], False)
                prev_av = (b, s)
                it += 1
            for j in range(it, len(jobs)):
                jobs[j]()
            job_select()
            for b in range(NBLK):
                job_full(b)
            emit_av(*prev_av, False, True)

            # ---- readout
            nc.scalar.copy(out_sb[:], ov8[:])
            for b in range(NBLK):
                isl = slice(b * IBLK, (b + 1) * IBLK)
                nc.sync.dma_start(out[:, isl], out_sb[2 * b:2 * b + 2, :])

    nc.compile()
    return nc


def kernel(x1, x2, x3, x4, Wq_w, Wq_b, Wk_w, Wk_b):
    from concourse.bass_utils import run_bass_kernel_spmd

    global LAST_RESULTS

    xs = [np.asarray(a, dtype=np.float32)[0, 0] for a in (x1, x2, x3, x4)]
    Wq_w = np.asarray(Wq_w, dtype=np.float32)
    Wq_b = np.asarray(Wq_b, dtype=np.float32)
    Wk_w = np.asarray(Wk_w, dtype=np.float32)
    Wk_b = np.asarray(Wk_b, dtype=np.float32)

    t1 = xs[0][:, -1]
    t2s = [x[:, -1] for x in xs]

    # ---- per-core full/boundary classification (exact, from timestamps)
    FJ = {}  # (m, p) -> (F[b], J[b])
    NBB = [1] * NBLK
    for p in range(2):
        qoff = NQ * p
        for m in range(M):
            F, J = [], []
            for b in range(NBLK):
                lo = t1[qoff + b * IBLK]
                hi = t1[qoff + b * IBLK + IBLK - 1]
                nfull = int(np.searchsorted(t2s[m], lo, side="right"))
                nvis = int(np.searchsorted(t2s[m], hi, side="right"))
                F.append(nfull // CHUNK)
                J.append(-(-nvis // CHUNK))
                NBB[b] = max(NBB[b], J[b] - F[b])
            FJ[(m, p)] = (F, J)

    nc = _build_program(NBB)

    NBSLOT = sum(NBB)
    boff = [sum(NBB[:b]) for b in range(NBLK)]

    # ---- host packing
    def blockdiag(Wl):
        b = np.zeros((128, 128), np.float32)
        b[:64, :64] = Wl
        b[64:, 64:] = Wl
        return b

    # Q weights: layers 0,1 blockdiag; final as [[W,W],[0,0]] and [[0,0],[W,W]]
    wq_h = np.zeros((4, 128, 128), np.float32)
    for l in range(NLIN - 1):
        wq_h[l] = blockdiag(Wq_w[l])
    wq_h[2, :64, :64] = Wq_w[2]
    wq_h[2, :64, 64:] = Wq_w[2]
    wq_h[3, 64:, :64] = Wq_w[2]
    wq_h[3, 64:, 64:] = Wq_w[2]
    wq_h = np.ascontiguousarray(wq_h.transpose(1, 0, 2).reshape(128, 4 * 128))
    bq_h = np.tile(Wq_b.T, (2, 1))  # [128, 3]
    bq_h = np.ascontiguousarray(
        np.concatenate([bq_h, bq_h[:, 2:3]], axis=1)
    )  # [128, 4]

    probe_h = np.ascontiguousarray(
        np.concatenate([np.eye(64, dtype=np.float32)] * 2, axis=0)
    )  # [128, 64]

    x1T = np.ascontiguousarray(xs[0].T)

    def pack_tile(xrows):
        """[128, D] key rows -> ([128, 64] xkT block, [128, 2] V, [128] t2)."""
        ev, od = xrows[0:64], xrows[64:128]
        blk = np.concatenate([ev.T, od.T], axis=0)  # [128, 64]
        v = np.concatenate([ev[:, 0:2], od[:, 0:2]], axis=0)  # [128, 2]
        tt = np.concatenate([ev[:, -1], od[:, -1]], axis=0)  # [128]
        return blk, v, tt

    in_maps = []
    for c in range(8):
        m, p = c // 2, c % 2
        xm = xs[m]
        qoff = NQ * p
        F, J = FJ[(m, p)]

        NSLOT = NPAIR + NBSLOT
        xkT_h = np.zeros((128, NSLOT * 64), np.float32)
        xkvG_h = np.zeros((128, NPAIR, 2), np.float32)
        xkvB_h = np.zeros((128, max(NBSLOT, 1), 8), np.float32)
        xt2b_h = np.full((128, max(NBSLOT, 1)), 1e30, np.float32)
        for j in range(NPAIR):
            blk, v, tt = pack_tile(xm[CHUNK * j:CHUNK * (j + 1)])
            xkT_h[:, 64 * j:64 * (j + 1)] = blk
            xkvG_h[:, j] = v
        for b in range(NBLK):
            for s in range(NBB[b]):
                t = F[b] + s
                slot = NPAIR + boff[b] + s
                if t < J[b]:
                    blk, v, tt = pack_tile(xm[CHUNK * t:CHUNK * (t + 1)])
                    xkT_h[:, 64 * slot:64 * (slot + 1)] = blk
                    xkvB_h[:, boff[b] + s, 2 * b:2 * b + 2] = v
                    xt2b_h[:, boff[b] + s] = tt
        step_h = np.zeros((NPAIR, NBLK), np.float32)
        if not os.environ.get('DBG_NO_FULL'):
            for b in range(NBLK):
                step_h[: F[b], b] = 1.0

        wk_h = np.stack([blockdiag(Wk_w[m][l]) for l in range(NLIN)])
        wk_h = np.ascontiguousarray(wk_h.transpose(1, 0, 2).reshape(128, NLIN * 128))
        bk_h = np.ascontiguousarray(np.tile(Wk_b[m].T, (2, 1)))  # [128, 3]

        # query-side: contiguous half, [first 1024 | second 1024] stacking
        xq = x1T[:, qoff:qoff + NQ]  # [64, 2048]
        xqT_h = np.concatenate([xq[:, : NQ // 2], xq[:, NQ // 2:]], axis=0)

        in_maps.append(
            {
                "xqT": np.ascontiguousarray(xqT_h).astype(BF16),
                "xkT": xkT_h.astype(BF16),
                "xkvG": np.ascontiguousarray(xkvG_h.reshape(128, NPAIR * 2)).astype(BF16),
                "xkvB": np.ascontiguousarray(
                    xkvB_h.reshape(128, max(NBSLOT, 1) * 8)).astype(BF16),
                "xt2b": xt2b_h,
                "t1p": np.ascontiguousarray(t1[qoff:qoff + NQ][None, :]),
                "probe": probe_h.astype(BF16),
                "stepm": step_h.astype(BF16),
                "wq": wq_h.astype(BF16),
                "bq": bq_h,
                "wk": wk_h.astype(BF16),
                "bk": bk_h,
            }
        )

    res = run_bass_kernel_spmd(nc, in_maps, core_ids=list(range(8)))
    LAST_RESULTS = res

    # ---- gather: sum over modalities per contiguous half, transpose
    acc = np.zeros((2, T), dtype=np.float32)
    for c in range(8):
        m, p = c // 2, c % 2
        acc[:, NQ * p:NQ * (p + 1)] += res.results[c]["out"]
    return np.ascontiguousarray(acc.T)[None]
